# revision 26
# baseline (speedup 1.0000x reference)
"""Trainium2 Bass kernel for one transformer block (B=2, T=2048, C=768, H=12,
inner=3072, fp32 I/O, causal, post-norm residual).

Sharding: 8 cores, token-interleaved. Core c handles batch c//4, tokens
p::4 (p = c%4) of that batch — every core runs the IDENTICAL program
(SPMD); causality is data-driven via per-core mask tensors.

v3 (vs 292us baseline): fused A' pipeline — scores+exp+ctx interleaved
INTO the K/V token-block loop so the ACT engine's ~60us of exp work (the
old phase-B bottleneck) hides under QKV matmuls:
- per tb: k, v computed; scores for the PREVIOUS tb's chunks run
  interleaved per head (score tiles -> exp -> boundary masks on the Pool
  engine -> ctx partial matmuls -> DVE accumulate into per-head f32
  SBUF accumulators ctxa [65,TQ]). PSUM: kv ring2 (2 banks) + v-ps2 (1)
  + score [128,1024] ring2 (4) + ctx [65,512] ring1 (1) = 8 exactly.
- q is kc-outer (6 psum banks, before the tb loop) with per-chunk
  wq/xtqh DMAs so the first matmul fires after ~330KB lands (was ~2MB).
- bv folded host-side into bo' = bo + bv @ Wo (ctx_norm = ctx*rcp
  absorbs +bv exactly via the denominator row): v eviction is a pure
  copy, no bias tile.
- softmax reciprocal = exp(-ln(dn)) on ACT (idle in B'), per head; the
  old 14.6us DVE reciprocal is gone. K=65 selector matmul broadcasts
  rcp; the psum->sbuf ctx eviction fuses into the normalize multiply.
- wk packed per-mc-slab so k(mc) streams as slabs land; wv split
  512/256; w1/w2 + xtq stream during B' when qz/kT/v free up.
"""

import sys

if "/opt/trn_rl_repo" not in sys.path:
    sys.path.insert(0, "/opt/trn_rl_repo")

import numpy as np
import ml_dtypes

import concourse.bacc as bacc
import concourse.mybir as mybir
import concourse.tile as tile
from concourse.bass_utils import run_bass_kernel_spmd

F32 = mybir.dt.float32
F32R = mybir.dt.float32r
BF16 = mybir.dt.bfloat16
ACTF = mybir.ActivationFunctionType

B, T, C = 2, 2048, 768
H, DH = 12, 64
IN = 3072
CC = C // 128          # 6 channel chunks
TBN = T // 512         # 4 token blocks of full seq
TQ = 512               # tokens per core
KCN = T // 128         # 16 k-chunks
ICN = IN // 128        # 24 inner chunks
EPS = 1e-4
SCALE = 1.0 / np.sqrt(DH)

# param pack order in "prk" [128, CC, 8]
P_BQ, P_BK, P_BO, P_B2, P_L1S, P_L1B, P_L2S, P_L2B = range(8)

# score-tile packing per tb: tiles of (global kc chunk, col offset).
# Chunk kc covers queries 32*kc..TQ (width TQ-32*kc). Offsets chosen so
# NO matmul output crosses a 2KB psum bank boundary (512 f32 cols) —
# a crossing write half-overwrites / half-accumulates-onto-stale.
# Gaps between chunks get exp'd (garbage, never read) — harmless.
SPACK = [
    [[(0, 0), (1, 512)], [(2, 0), (3, 512)]],
    [[(4, 0), (5, 512)], [(6, 0), (7, 512)]],
    [[(8, 0), (9, 256), (10, 512), (11, 704)]],
    [[(12, 0), (13, 128), (14, 224), (15, 288)]],
]


def _build_nc():
    nc = bacc.Bacc("TRN2", target_bir_lowering=False, debug=False,
                   enable_asserts=False, num_devices=8)
    d = {}
    d["xt"] = nc.dram_tensor("xt", [128, TBN, CC, 512], BF16,
                             kind="ExternalInput").ap()
    d["xtqh"] = nc.dram_tensor("xtqh", [128, CC, TQ], BF16,
                               kind="ExternalInput").ap()
    d["xtq"] = nc.dram_tensor("xtq", [128, CC, TQ], F32R,
                              kind="ExternalInput").ap()
    d["wq"] = nc.dram_tensor("wq", [128, CC, C], BF16,
                             kind="ExternalInput").ap()
    d["wk"] = nc.dram_tensor("wk", [128, CC, CC, 128], BF16,
                             kind="ExternalInput").ap()  # [p, mc, kc, j]
    d["wva"] = nc.dram_tensor("wva", [128, CC, 512], BF16,
                              kind="ExternalInput").ap()
    d["wvb"] = nc.dram_tensor("wvb", [128, CC, 256], BF16,
                              kind="ExternalInput").ap()
    d["wo"] = nc.dram_tensor("wo", [128, CC, C], BF16,
                             kind="ExternalInput").ap()
    d["w1"] = nc.dram_tensor("w1", [128, 4, CC, C], BF16,
                             kind="ExternalInput").ap()
    d["w2"] = nc.dram_tensor("w2", [128, 4, CC, C], BF16,
                             kind="ExternalInput").ap()
    d["prk"] = nc.dram_tensor("prk", [128, CC, 8], F32, kind="ExternalInput").ap()
    d["b1p"] = nc.dram_tensor("b1p", [128, ICN], F32, kind="ExternalInput").ap()
    d["msk"] = nc.dram_tensor("msk", [128, 32], BF16, kind="ExternalInput").ap()
    d["ones"] = nc.dram_tensor("ones", [128, H], F32R, kind="ExternalInput").ap()
    d["lnz"] = nc.dram_tensor("lnz", [2, 65, TQ], F32R, kind="ExternalInput").ap()
    d["selm"] = nc.dram_tensor("selm", [65, 3, DH], F32R, kind="ExternalInput").ap()
    d["lnt"] = nc.dram_tensor("lnt", [2, 65, C], F32R, kind="ExternalInput").ap()
    d["outT"] = nc.dram_tensor("outT", [C, TQ], F32, kind="ExternalOutput").ap()

    with tile.TileContext(nc) as tc:
        _emit(nc, tc, d)
    nc.finalize()
    return nc


def _ln_bcast(nc, pool, eps_sb, lnr1, lnr2, mean, ex2, tagp):
    """Fill lnr1 (row0 = istd) and lnr2 (row0 = mean*istd, row32 = ones)
    so the LN broadcasts run as K=65 selector matmuls at full PE rate.
    istd = exp(-0.5*ln(var+eps)) on ACT."""
    n = float(C)
    m2 = pool.tile([1, TQ], F32, name="ln_m2", tag=tagp + "m2")
    nc.vector.tensor_mul(m2[:], mean[:], mean[:])
    dv = pool.tile([1, TQ], F32, name="ln_d", tag=tagp + "d")
    nc.vector.tensor_sub(dv[:], ex2[:], m2[:])
    lnv = pool.tile([1, TQ], F32, name="ln_lnv", tag=tagp + "lnv")
    nc.scalar.activation(lnv[:], dv[:], ACTF.Ln,
                         scale=n / (n - 1.0), bias=eps_sb[:])
    with nc.allow_low_precision(reason="f32r matmul operand"):
        nc.scalar.activation(lnr1[0:1, :], lnv[:], ACTF.Exp, scale=-0.5)
        nc.vector.tensor_mul(lnr2[0:1, :], mean[:], lnr1[0:1, :])


def _emit(nc, tc, d):
    # ---- persistent constants ------------------------------------------
    const = tc.alloc_tile_pool(name="const", bufs=1, side="left")
    onesh_sb = const.tile([128, H], F32R, name="onesh_sb")
    selm_sb = const.tile([65, 3, DH], F32R, name="selm_sb")
    lnt1_sb = const.tile([65, C], F32R, name="lnt1_sb")
    lnt2_sb = const.tile([65, C], F32R, name="lnt2_sb")
    eps_sb = const.tile([1, 1], F32, name="eps_sb")
    prk_sb = const.tile([128, CC, 8], F32, name="prk_sb")
    b1p_sb = const.tile([128, ICN], F32, name="b1p_sb")
    mskb_sb = const.tile([128, 32], BF16, name="mskb_sb")

    def prm(cc, pi):
        return prk_sb[:, cc, pi].unsqueeze(-1)  # [128,1]

    nc.vector.memset(eps_sb[:], float(EPS))
    nc.gpsimd.dma_start(out=onesh_sb[:], in_=d["ones"][:])
    ones1_sb = onesh_sb[:, 0:1]
    nc.gpsimd.dma_start(out=prk_sb[:], in_=d["prk"][:])
    nc.gpsimd.dma_start(out=mskb_sb[:], in_=d["msk"][:])
    nc.gpsimd.dma_start(out=selm_sb[:], in_=d["selm"][:])
    nc.gpsimd.dma_start(out=lnt1_sb[:], in_=d["lnt"][0, :, :])
    nc.gpsimd.dma_start(out=lnt2_sb[:], in_=d["lnt"][1, :, :])
    nc.gpsimd.dma_start(out=b1p_sb[:], in_=d["b1p"][:])

    # ---- persistent activation tensors ---------------------------------
    # left-side stack order = reverse release order: ctxT/ctxa released
    # late, qT/kv right after A'
    ctxT_pool = tc.alloc_tile_pool(name="ctxTp", bufs=1, side="left")
    ctxT_sb = [ctxT_pool.tile([128, TQ], BF16, name=f"ctxT{cc}")
               for cc in range(CC)]
    ctxa_pool = tc.alloc_tile_pool(name="ctxap", bufs=1, side="left")
    ctxa_sb = [ctxa_pool.tile([DH + 1, TQ], F32, name=f"ctxa{h}")
               for h in range(H)]
    qT_pool = tc.alloc_tile_pool(name="qTp", bufs=1, side="left")
    qz_sb = [qT_pool.tile([128, TQ], BF16, name=f"qz{h}") for h in range(H)]
    for h in range(H):
        ro = (h % 2) * DH
        with nc.allow_low_precision(reason="zero fill"):
            nc.vector.memset(qz_sb[h][(DH - ro):(128 - ro), :], 0.0)

    kv_pool = tc.alloc_tile_pool(name="kvp", bufs=1, side="left")
    kT_sb = [kv_pool.tile([128, T], BF16, name=f"kT{cc}") for cc in range(CC)]
    v_sb = [kv_pool.tile([128, H, DH + 1], BF16, name=f"v{tch}")
            for tch in range(KCN)]
    for tch in range(KCN):
        with nc.allow_low_precision(reason="ones fill"):
            nc.vector.tensor_copy(v_sb[tch][:, :, DH], onesh_sb[:])

    # ==================== phase A': QKV + scores + exp + ctx ============
    wo_pool = tc.alloc_tile_pool(name="wop", bufs=1, side="right")
    with tc.tile_pool(name="xqs", bufs=1, side="right") as xq_pool, \
         tc.tile_pool(name="wqs", bufs=1, side="right") as wq_pool, \
         tc.tile_pool(name="wkvs", bufs=1, side="right") as wkv_pool, \
         tc.tile_pool(name="xts", bufs=2, side="right") as xt_pool, \
         tc.tile_pool(name="etp", bufs=1, side="right") as et_pool:

        # -- DMA schedule ------------------------------------------------
        # sync q: xtqh chunks, xt blocks, wva/wvb
        # scalar q: wq chunks, wk mc-slabs
        # gpsimd q: small consts (above), wo
        xtqh_pk = xq_pool.tile([128, CC, TQ], BF16, name="xtqh_pk")
        wq_pk = wq_pool.tile([128, CC, C], BF16, name="wq_pk")
        for kc in range(CC):
            nc.sync.dma_start(out=xtqh_pk[:, kc, :], in_=d["xtqh"][:, kc, :])
            nc.scalar.dma_start(out=wq_pk[:, kc, :], in_=d["wq"][:, kc, :])
        wk_pk = wkv_pool.tile([128, CC, CC, 128], BF16, name="wk_pk")
        for mc in range(CC):
            nc.scalar.dma_start(out=wk_pk[:, mc, :, :],
                                in_=d["wk"][:, mc, :, :])
        xt_tiles = {}
        for tb in range(2):
            t = xt_pool.tile([128, CC, 512], BF16, name="xt_t", tag="xt")
            nc.sync.dma_start(out=t[:], in_=d["xt"][:, tb, :, :])
            xt_tiles[tb] = t
        wva_pk = wkv_pool.tile([128, CC, 512], BF16, name="wva_pk")
        nc.sync.dma_start(out=wva_pk[:], in_=d["wva"][:])
        wvb_pk = wkv_pool.tile([128, CC, 256], BF16, name="wvb_pk")
        nc.sync.dma_start(out=wvb_pk[:], in_=d["wvb"][:])
        wo_pk = wo_pool.tile([128, CC, C], BF16, name="wo_pk")
        nc.gpsimd.dma_start(out=wo_pk[:], in_=d["wo"][:])
        wo_sb = [wo_pk[:, cc, :] for cc in range(CC)]

        # -- q: kc-outer so first matmul needs only chunk 0 of wq/xtqh --
        with tc.tile_pool(name="pq", bufs=1, space="PSUM") as pq:
            ps_q = [pq.tile([128, TQ], F32, name=f"psq{mc}", tag=f"q{mc}")
                    for mc in range(CC)]
            for kc in range(CC):
                for mc in range(CC):
                    nc.tensor.matmul(ps_q[mc][:],
                                     wq_pk[:, kc, mc * 128:(mc + 1) * 128],
                                     xtqh_pk[:, kc, :],
                                     start=(kc == 0), stop=(kc == CC - 1))
            for mc in range(CC):
                with nc.allow_low_precision(reason="bf16 activations"):
                    nc.vector.tensor_scalar_add(
                        qz_sb[2 * mc][0:DH, :], ps_q[mc][0:DH, :],
                        prm(mc, P_BQ)[0:DH, :])
                    nc.vector.tensor_scalar_add(
                        qz_sb[2 * mc + 1][DH:128, :], ps_q[mc][DH:128, :],
                        prm(mc, P_BQ)[DH:128, :])

        with tc.tile_pool(name="pkv", bufs=1, space="PSUM") as pkv, \
             tc.tile_pool(name="psc", bufs=1, space="PSUM") as psc, \
             tc.tile_pool(name="pctx", bufs=1, space="PSUM") as pctx:

            def emit_k(tb, mc):
                xt_blk = xt_tiles[tb]
                ps = pkv.tile([128, 512], F32, name="ps_k", tag="kv1",
                              bufs=2)
                for kc in range(CC):
                    nc.tensor.matmul(
                        ps[:], wk_pk[:, mc, kc, :], xt_blk[:, kc, :],
                        start=(kc == 0), stop=(kc == CC - 1))
                with nc.allow_low_precision(reason="bf16 activations"):
                    nc.vector.tensor_scalar_add(
                        kT_sb[mc][:, tb * 512:(tb + 1) * 512], ps[:],
                        prm(mc, P_BK))

            def emit_v(tb, tci):
                xt_blk = xt_tiles[tb]
                tch = tb * 4 + tci
                ps1 = pkv.tile([128, 512], F32, name="ps_v1", tag="kv1",
                               bufs=2)
                ps2 = pkv.tile([128, 256], F32, name="ps_v2", tag="v2",
                               bufs=1)
                for kc in range(CC):
                    xsl = xt_blk[:, kc, tci * 128:(tci + 1) * 128]
                    nc.tensor.matmul(ps1[:], xsl, wva_pk[:, kc, :],
                                     start=(kc == 0), stop=(kc == CC - 1))
                    nc.tensor.matmul(ps2[:], xsl, wvb_pk[:, kc, :],
                                     start=(kc == 0), stop=(kc == CC - 1))
                vt = v_sb[tch]
                with nc.allow_low_precision(reason="bf16 activations"):
                    nc.vector.tensor_copy(
                        vt[:, 0:8, 0:DH],
                        ps1[:].rearrange("p (h d) -> p h d", d=DH))
                    nc.vector.tensor_copy(
                        vt[:, 8:H, 0:DH],
                        ps2[:].rearrange("p (h d) -> p h d", d=DH))

            cur_et = {}   # h -> list of (et_tile, off, w, kc) for this tb

            def emit_scores(tb, h):
                cc = h // 2
                kTh = kT_sb[cc]
                qzh = qz_sb[h]
                lst = []
                for kcs in SPACK[tb]:
                    tw = kcs[-1][1] + TQ - 32 * kcs[-1][0]
                    ps = psc.tile([128, 1024], F32, name="ps_s",
                                  tag="sA", bufs=2)
                    for kc, off in kcs:
                        nc.tensor.matmul(
                            ps[:, off:off + TQ - 32 * kc],
                            kTh[:, kc * 128:(kc + 1) * 128],
                            qzh[:, 32 * kc:TQ],
                            start=True, stop=True)
                    et = et_pool.tile([128, 1024], BF16, name="et",
                                      tag="et", bufs=6)
                    nc.scalar.activation(et[:, 0:tw], ps[:, 0:tw], ACTF.Exp,
                                         scale=float(SCALE))
                    for kc, o in kcs:
                        with nc.allow_low_precision(reason="bf16 mask"):
                            nc.gpsimd.tensor_mul(
                                et[:, o:o + 32], et[:, o:o + 32],
                                mskb_sb[:])
                        lst.append((et, o, TQ - 32 * kc, kc))
                cur_et[h] = lst

            def emit_ctx(tb, h):
                ps = pctx.tile([DH + 1, TQ], F32, name="ctx_ps", tag="ctx",
                               bufs=1)
                lst = cur_et.pop(h)
                w0 = lst[0][2]
                for i, (et, o, w, kc) in enumerate(lst):
                    nc.tensor.matmul(ps[:, TQ - w:], v_sb[kc][:, h, :],
                                     et[:, o:o + w],
                                     start=(i == 0), stop=(i == len(lst) - 1))
                if tb == 0:
                    nc.vector.tensor_copy(ctxa_sb[h][:], ps[:])
                else:
                    nc.vector.tensor_add(ctxa_sb[h][:, TQ - w0:],
                                         ctxa_sb[h][:, TQ - w0:],
                                         ps[:, TQ - w0:])

            # tb0 k/v (nothing to interleave yet)
            for mc in range(CC):
                emit_k(0, mc)
            for tci in range(4):
                emit_v(0, tci)

            # windows: scores/exp/ctx of tb interleaved with k/v of tb+1
            for stb in range(TBN):
                ftb = stb + 1
                fills = []
                if ftb < TBN:
                    if ftb + 1 < TBN and (ftb + 1) not in xt_tiles:
                        t2 = xt_pool.tile([128, CC, 512], BF16,
                                          name="xt_t", tag="xt")
                        nc.sync.dma_start(out=t2[:],
                                          in_=d["xt"][:, ftb + 1, :, :])
                        xt_tiles[ftb + 1] = t2
                    fills += [(emit_k, (ftb, mc)) for mc in range(CC)]
                    fills += [(emit_v, (ftb, tci)) for tci in range(4)]
                for h in range(H):
                    emit_scores(stb, h)
                    if h > 0:
                        emit_ctx(stb, h - 1)
                    if h < len(fills):
                        fn, args = fills[h]
                        fn(*args)
                emit_ctx(stb, H - 1)

    kv_pool.release()
    qT_pool.release()

    # ==================== phase B': norm + Wo + LN1 =====================
    w1pool = tc.alloc_tile_pool(name="w1pool", bufs=2, side="right")
    w2pool = tc.alloc_tile_pool(name="w2pool", bufs=2, side="right")
    xtq_pool = tc.alloc_tile_pool(name="xtqp", bufs=1, side="right")

    hT_holder = {}
    with tc.tile_pool(name="bpool", bufs=2, side="right") as bpool, \
         tc.tile_pool(name="r1pool", bufs=1, side="right") as r1pool:

        # streams that waited on qz/kT/v SBUF space
        xtq_pk = xtq_pool.tile([128, CC, TQ], F32R, name="xtq_pk")
        nc.gpsimd.dma_start(out=xtq_pk[:], in_=d["xtq"][:])
        xtq_sb = [xtq_pk[:, cc, :] for cc in range(CC)]
        w1blk = {}
        for jb in range(2):
            t = w1pool.tile([128, CC, C], BF16, name="w1_t", tag="w1")
            nc.gpsimd.dma_start(out=t[:], in_=d["w1"][:, jb, :, :])
            w1blk[jb] = [t[:, kc, :] for kc in range(CC)]
        w2g = {}
        for gg in range(2):
            t = w2pool.tile([128, CC, C], BF16, name="w2_t", tag="w2")
            nc.sync.dma_start(out=t[:], in_=d["w2"][:, gg, :, :])
            w2g[gg] = t

        # softmax normalization: rcp = exp(-ln(dn)) per head on ACT,
        # broadcast via K=65 selector matmul, fused normalize into the
        # ctxa -> ctxT eviction multiply.
        dn3 = bpool.tile([65, TQ], F32R, name="dn3", tag="dn3", bufs=1)
        nc.gpsimd.dma_start(out=dn3[:], in_=d["lnz"][0, :, :])
        with tc.tile_pool(name="ppb", bufs=1, space="PSUM") as ppb:
            for h in range(H):
                cc, ro, j = h // 2, (h % 2) * DH, h % 3
                lntmp = bpool.tile([1, TQ], F32, name="lntmp", tag="lnt")
                nc.scalar.activation(lntmp[:], ctxa_sb[h][DH:DH + 1, :],
                                     ACTF.Ln)
                with nc.allow_low_precision(reason="f32r matmul operand"):
                    nc.scalar.activation(dn3[32 * j:32 * j + 1, :],
                                         lntmp[:], ACTF.Exp, scale=-1.0)
                pb = ppb.tile([DH, TQ], F32, name="pb", tag="pb", bufs=2)
                nc.tensor.matmul(pb[:], selm_sb[:, j, :], dn3[:],
                                 start=True, stop=True)
                with nc.allow_low_precision(reason="bf16 activations"):
                    nc.vector.tensor_mul(ctxT_sb[cc][ro:ro + DH, :],
                                         ctxa_sb[h][0:DH, :], pb[:])

        # Wo + residual + LN1 stats
        r1_sb = [r1pool.tile([128, TQ], F32R, name=f"r1{cc}")
                 for cc in range(CC)]
        lnr1 = bpool.tile([65, TQ], F32R, name="lnr1", tag="lnr1")
        lnr2 = bpool.tile([65, TQ], F32R, name="lnr2", tag="lnr2")
        nc.gpsimd.dma_start(out=lnr1[:], in_=d["lnz"][0, :, :])
        nc.gpsimd.dma_start(out=lnr2[:], in_=d["lnz"][1, :, :])
        with tc.tile_pool(name="pao", bufs=2, space="PSUM") as pao, \
             tc.tile_pool(name="pst", bufs=2, space="PSUM") as pst:
            ps_sum = pst.tile([1, TQ], F32, name="ps_sum", tag="st")
            ps_sq = pst.tile([1, TQ], F32, name="ps_sq", tag="st")
            for mc in range(CC):
                ps = pao.tile([128, TQ], F32, name="ps_ao", tag="ao")
                for kc in range(CC):
                    nc.tensor.matmul(ps[:],
                                     wo_sb[kc][:, mc * 128:(mc + 1) * 128],
                                     ctxT_sb[kc][:],
                                     start=(kc == 0), stop=(kc == CC - 1))
                nc.vector.scalar_tensor_tensor(
                    r1_sb[mc][:], ps[:], prm(mc, P_BO), xtq_sb[mc][:],
                    mybir.AluOpType.add, mybir.AluOpType.add)
                nc.tensor.matmul(ps_sum[:], ones1_sb[:], r1_sb[mc][:],
                                 start=(mc == 0), stop=(mc == CC - 1))
                sq = bpool.tile([128, TQ], F32R, name="sq", tag="sq")
                nc.scalar.activation(sq[:], r1_sb[mc][:], ACTF.Square)
                nc.tensor.matmul(ps_sq[:], ones1_sb[:], sq[:],
                                 start=(mc == 0), stop=(mc == CC - 1))
            n = float(C)
            mean1 = bpool.tile([1, TQ], F32R, name="l1mean", tag="l1mean")
            nc.scalar.activation(mean1[:], ps_sum[:], ACTF.Copy, scale=1.0 / n)
            ex21 = bpool.tile([1, TQ], F32, name="l1ex2", tag="l1ex2")
            nc.scalar.activation(ex21[:], ps_sq[:], ACTF.Copy, scale=1.0 / n)
        hT_pool = tc.alloc_tile_pool(name="hTp", bufs=1, side="left")
        hT_sb = [hT_pool.tile([128, TQ], BF16, name=f"hT{cc}")
                 for cc in range(CC)]
        hT_holder["pool"] = hT_pool
        _ln_bcast(nc, bpool, eps_sb, lnr1, lnr2, mean1, ex21, "l1")
        with tc.tile_pool(name="pbc2", bufs=2, space="PSUM") as pbc2:
            for cc in range(CC):
                csl = slice(cc * 128, (cc + 1) * 128)
                pb2 = pbc2.tile([128, 2 * TQ], F32, name="lnpb", tag="bc")
                nc.tensor.matmul(pb2[:, 0:TQ], lnt1_sb[:, csl],
                                 lnr1[:], start=True, stop=True)
                nc.tensor.matmul(pb2[:, TQ:], lnt1_sb[:, csl],
                                 lnr2[:], start=True, stop=True)
                t1 = bpool.tile([128, TQ], F32, name="ln_t1", tag="lnt1")
                nc.vector.tensor_mul(t1[:], r1_sb[cc][:], pb2[:, 0:TQ])
                with nc.allow_low_precision(reason="bf16 activations"):
                    nc.vector.tensor_sub(hT_sb[cc][:], t1[:], pb2[:, TQ:])

    xtq_pool.release()

    # ==================== phase D: MLP + residual + LN2 =================
    with tc.tile_pool(name="dpool", bufs=2, side="right") as dpool, \
         tc.tile_pool(name="r2pool", bufs=1, side="right") as r2pool:

        r2_sb = [r2pool.tile([128, TQ], F32R, name=f"r2{cc}")
                 for cc in range(CC)]
        lnr1b = dpool.tile([65, TQ], F32R, name="lnr1b", tag="lnr1b")
        lnr2b = dpool.tile([65, TQ], F32R, name="lnr2b", tag="lnr2b")
        nc.gpsimd.dma_start(out=lnr1b[:], in_=d["lnz"][0, :, :])
        nc.gpsimd.dma_start(out=lnr2b[:], in_=d["lnz"][1, :, :])
        with tc.tile_pool(name="pfc2", bufs=1, space="PSUM") as pfc2:
            ps_m = [pfc2.tile([128, TQ], F32, name=f"ps_m{mc}", tag=f"m{mc}")
                    for mc in range(CC)]
            with tc.tile_pool(name="pfc1", bufs=2, space="PSUM") as pfc1:
                for kc2 in range(ICN):
                    jb = kc2 // CC
                    ps1 = pfc1.tile([128, TQ], F32, name="ps1", tag="f1")
                    co = (kc2 % CC) * 128
                    for kc in range(CC):
                        nc.tensor.matmul(
                            ps1[:], w1blk[jb][kc][:, co:co + 128],
                            hT_sb[kc][:],
                            start=(kc == 0), stop=(kc == CC - 1))
                    g = dpool.tile([128, TQ], BF16, name="g", tag="g")
                    with nc.allow_low_precision(reason="bf16 activations"):
                        nc.scalar.activation(g[:], ps1[:],
                                             ACTF.Gelu_apprx_tanh,
                                             bias=b1p_sb[:, kc2].unsqueeze(-1))
                    w2t = w2g[kc2 // CC][:, kc2 % CC, :]
                    for mc in range(CC):
                        nc.tensor.matmul(ps_m[mc][:],
                                         w2t[:, mc * 128:(mc + 1) * 128],
                                         g[:], start=(kc2 == 0),
                                         stop=(kc2 == ICN - 1))
                    # ring prefetches: issued after this iteration's readers
                    if kc2 % CC == CC - 1 and jb + 2 <= 3:
                        t = w1pool.tile([128, CC, C], BF16, name="w1_t",
                                        tag="w1")
                        nc.gpsimd.dma_start(out=t[:],
                                            in_=d["w1"][:, jb + 2, :, :])
                        w1blk[jb + 2] = [t[:, kc, :] for kc in range(CC)]
                        t2 = w2pool.tile([128, CC, C], BF16, name="w2_t",
                                         tag="w2")
                        nc.sync.dma_start(out=t2[:],
                                          in_=d["w2"][:, jb + 2, :, :])
                        w2g[jb + 2] = t2
            with tc.tile_pool(name="pst2", bufs=2, space="PSUM") as pst2:
                ps_sum2 = pst2.tile([1, TQ], F32, name="ps_sum2", tag="st")
                ps_sq2 = pst2.tile([1, TQ], F32, name="ps_sq2", tag="st")
                for mc in range(CC):
                    nc.vector.scalar_tensor_tensor(
                        r2_sb[mc][:], ps_m[mc][:], prm(mc, P_B2),
                        hT_sb[mc][:], mybir.AluOpType.add,
                        mybir.AluOpType.add)
                    nc.tensor.matmul(ps_sum2[:], ones1_sb[:], r2_sb[mc][:],
                                     start=(mc == 0), stop=(mc == CC - 1))
                    sq = dpool.tile([128, TQ], F32R, name="sq2", tag="sq")
                    nc.scalar.activation(sq[:], r2_sb[mc][:], ACTF.Square)
                    nc.tensor.matmul(ps_sq2[:], ones1_sb[:], sq[:],
                                     start=(mc == 0), stop=(mc == CC - 1))
                n = float(C)
                mean2 = dpool.tile([1, TQ], F32R, name="l2mean", tag="l2mean")
                nc.scalar.activation(mean2[:], ps_sum2[:], ACTF.Copy,
                                     scale=1.0 / n)
                ex22 = dpool.tile([1, TQ], F32, name="l2ex2", tag="l2ex2")
                nc.scalar.activation(ex22[:], ps_sq2[:], ACTF.Copy,
                                     scale=1.0 / n)
        hT_holder["pool"].release()
        ctxa_pool.release()
        _ln_bcast(nc, dpool, eps_sb, lnr1b, lnr2b, mean2, ex22, "l2")
        with tc.tile_pool(name="pbc3", bufs=2, space="PSUM") as pbc3:
            for cc in range(CC):
                csl = slice(cc * 128, (cc + 1) * 128)
                pb3 = pbc3.tile([128, 2 * TQ], F32, name="lnpb3", tag="bc")
                nc.tensor.matmul(pb3[:, 0:TQ], lnt2_sb[:, csl],
                                 lnr1b[:], start=True, stop=True)
                nc.tensor.matmul(pb3[:, TQ:], lnt2_sb[:, csl],
                                 lnr2b[:], start=True, stop=True)
                t1 = dpool.tile([128, TQ], F32, name="ln_t13", tag="lnt13")
                nc.vector.tensor_mul(t1[:], r2_sb[cc][:], pb3[:, 0:TQ])
                ot = dpool.tile([128, TQ], F32, name=f"o{cc}", tag=f"o{cc}",
                                bufs=1)
                nc.vector.tensor_sub(ot[:], t1[:], pb3[:, TQ:])
                nc.sync.dma_start(out=d["outT"][cc * 128:(cc + 1) * 128, :],
                                  in_=ot[:])

    w2pool.release()
    w1pool.release()
    wo_pool.release()
    ctxT_pool.release()
    const.release()


_NC = None


def _get_nc():
    global _NC
    if _NC is None:
        _NC = _build_nc()
    return _NC


def _prep_inmaps(x, Wq, bq, Wk, bk, Wv, bv, Wo, bo, ln1_s, ln1_b,
                 W1, b1, W2, b2, ln2_s, ln2_b):
    f32 = np.float32
    bf16 = ml_dtypes.bfloat16

    def pk(a):
        # [A*128, c...] -> [128, A, c...] contiguous (partition-major)
        a = np.asarray(a)
        return np.ascontiguousarray(
            a.reshape(-1, 128, *a.shape[1:]).swapaxes(0, 1))

    xT = [np.ascontiguousarray(np.asarray(x)[b].T, dtype=f32)
          for b in range(B)]
    xTh = [xb.astype(bf16) for xb in xT]
    wq = pk(np.asarray(Wq, f32).astype(bf16))
    # wk packed per-mc-slab: [128, mc, kc, 128]
    wk_p = pk(np.asarray(Wk, f32).astype(bf16))        # [128, kc, 768]
    wk = np.ascontiguousarray(
        wk_p.reshape(128, CC, CC, 128).swapaxes(1, 2))
    wv_p = pk(np.asarray(Wv, f32).astype(bf16))
    wva = np.ascontiguousarray(wv_p[:, :, 0:512])
    wvb = np.ascontiguousarray(wv_p[:, :, 512:768])
    wo = pk(np.asarray(Wo, f32).astype(bf16))
    w1f = np.asarray(W1, f32).astype(bf16)
    w1 = np.stack([pk(w1f[:, jb * C:(jb + 1) * C]) for jb in range(4)],
                  axis=1)
    w2f = np.asarray(W2, f32).astype(bf16)
    w2 = np.stack([pk(w2f[gg * C:(gg + 1) * C, :]) for gg in range(4)],
                  axis=1)
    # fold bv into bo: ctx_norm = ctx*rcp absorbs +bv exactly
    bo_f = np.asarray(bo, f32) + np.asarray(bv, f32) @ np.asarray(Wo, f32)
    prk = np.zeros((128, CC, 8), f32)
    for pi, arr in ((P_BQ, bq), (P_BK, bk), (P_BO, bo_f), (P_B2, b2),
                    (P_L1S, ln1_s), (P_L1B, ln1_b), (P_L2S, ln2_s),
                    (P_L2B, ln2_b)):
        prk[:, :, pi] = np.asarray(arr, f32).reshape(CC, 128).T
    b1p = np.ascontiguousarray(np.asarray(b1, f32).reshape(ICN, 128).T)
    lnt = np.zeros((2, 65, C), f32)
    lnt[0, 0], lnt[0, 32] = np.asarray(ln1_s, f32), -np.asarray(ln1_b, f32)
    lnt[1, 0], lnt[1, 32] = np.asarray(ln2_s, f32), -np.asarray(ln2_b, f32)
    selm = np.zeros((65, 3, DH), f32)
    for j in range(3):
        selm[32 * j, j, :] = 1.0
    ones = np.ones((128, H), f32)
    lnz = np.zeros((2, 65, TQ), f32)
    lnz[1, 32, :] = 1.0

    kk = np.arange(128)[:, None]
    in_maps = []
    for c in range(8):
        b, p = c // 4, c % 4
        qq = np.arange(32)[None, :]
        msk = np.ascontiguousarray(
            (kk <= 4 * qq + p).astype(bf16))  # k <= 4j+p, all chunks

        xtp = pk(xTh[b])  # [128, CC, T]
        xtp = np.ascontiguousarray(
            xtp.reshape(128, CC, TBN, 512).swapaxes(1, 2))
        in_maps.append({
            "xt": xtp,
            "xtqh": pk(np.ascontiguousarray(xTh[b][:, p::4])),
            "xtq": pk(np.ascontiguousarray(xT[b][:, p::4])),
            "wq": wq, "wk": wk, "wva": wva, "wvb": wvb, "wo": wo,
            "w1": w1, "w2": w2,
            "prk": prk, "b1p": b1p, "msk": msk,
            "lnt": lnt, "selm": selm, "ones": ones, "lnz": lnz,
        })
    return in_maps


def _run(in_maps, trace=False, **kw):
    nc = _get_nc()
    return run_bass_kernel_spmd(nc, in_maps, list(range(8)), trace=trace, **kw)


def kernel(**inputs):
    in_maps = _prep_inmaps(**inputs)
    res = _run(in_maps)
    out = np.empty((B, T, C), np.float32)
    for c in range(8):
        b, p = c // 4, c % 4
        out[b, p::4, :] = res.results[c]["outT"].T
    return out


# revision 29
# speedup vs baseline: 1.0185x; 1.0185x over previous
"""Trainium2 Bass kernel for one transformer block (B=2, T=2048, C=768, H=12,
inner=3072, fp32 I/O, causal, post-norm residual).

Sharding: 8 cores, token-interleaved. Core c handles batch c//4, tokens
p::4 (p = c%4) of that batch — every core runs the IDENTICAL program
(SPMD); causality is data-driven via per-core mask tensors.

v3 (vs 292us baseline): fused A' pipeline — scores+exp+ctx interleaved
INTO the K/V token-block loop so the ACT engine's ~60us of exp work (the
old phase-B bottleneck) hides under QKV matmuls:
- per tb: k, v computed; scores for the PREVIOUS tb's chunks run
  interleaved per head (score tiles -> exp -> boundary masks on the Pool
  engine -> ctx partial matmuls -> DVE accumulate into per-head f32
  SBUF accumulators ctxa [65,TQ]). PSUM: kv ring2 (2 banks) + v-ps2 (1)
  + score [128,1024] ring2 (4) + ctx [65,512] ring1 (1) = 8 exactly.
- q is kc-outer (6 psum banks, before the tb loop) with per-chunk
  wq/xtqh DMAs so the first matmul fires after ~330KB lands (was ~2MB).
- bv folded host-side into bo' = bo + bv @ Wo (ctx_norm = ctx*rcp
  absorbs +bv exactly via the denominator row): v eviction is a pure
  copy, no bias tile.
- softmax reciprocal = exp(-ln(dn)) on ACT (idle in B'), per head; the
  old 14.6us DVE reciprocal is gone. K=65 selector matmul broadcasts
  rcp; the psum->sbuf ctx eviction fuses into the normalize multiply.
- wk packed per-mc-slab so k(mc) streams as slabs land; wv split
  512/256; w1/w2 + xtq stream during B' when qz/kT/v free up.
"""

import sys

if "/opt/trn_rl_repo" not in sys.path:
    sys.path.insert(0, "/opt/trn_rl_repo")

import numpy as np
import ml_dtypes

import concourse.bacc as bacc
import concourse.mybir as mybir
import concourse.tile as tile
from concourse.bass_utils import run_bass_kernel_spmd

F32 = mybir.dt.float32
F32R = mybir.dt.float32r
BF16 = mybir.dt.bfloat16
ACTF = mybir.ActivationFunctionType

B, T, C = 2, 2048, 768
H, DH = 12, 64
IN = 3072
CC = C // 128          # 6 channel chunks
TBN = T // 512         # 4 token blocks of full seq
TQ = 512               # tokens per core
KCN = T // 128         # 16 k-chunks
ICN = IN // 128        # 24 inner chunks
EPS = 1e-4
SCALE = 1.0 / np.sqrt(DH)

# param pack order in "prk" [128, CC, 8]
P_BQ, P_BK, P_BO, P_B2, P_L1S, P_L1B, P_L2S, P_L2B = range(8)

# score-tile packing per tb: tiles of (global kc chunk, col offset).
# Chunk kc covers queries 32*kc..TQ (width TQ-32*kc). Offsets chosen so
# NO matmul output crosses a 2KB psum bank boundary (512 f32 cols) —
# a crossing write half-overwrites / half-accumulates-onto-stale.
# Gaps between chunks get exp'd (garbage, never read) — harmless.
SPACK = [
    [[(0, 0), (1, 512)], [(2, 0), (3, 512)]],
    [[(4, 0), (5, 512)], [(6, 0), (7, 512)]],
    [[(8, 0), (9, 256), (10, 512), (11, 704)]],
    [[(12, 0), (13, 128), (14, 224), (15, 288)]],
]


def _build_nc():
    nc = bacc.Bacc("TRN2", target_bir_lowering=False, debug=False,
                   enable_asserts=False, num_devices=8)
    d = {}
    d["xt"] = nc.dram_tensor("xt", [128, TBN, CC, 512], BF16,
                             kind="ExternalInput").ap()
    d["xtqh"] = nc.dram_tensor("xtqh", [128, CC, TQ], BF16,
                               kind="ExternalInput").ap()
    d["xtq"] = nc.dram_tensor("xtq", [128, CC, TQ], F32R,
                              kind="ExternalInput").ap()
    d["wq"] = nc.dram_tensor("wq", [128, CC, C], BF16,
                             kind="ExternalInput").ap()
    d["wk"] = nc.dram_tensor("wk", [128, CC, CC, 128], BF16,
                             kind="ExternalInput").ap()  # [p, mc, kc, j]
    d["wva"] = nc.dram_tensor("wva", [128, CC, 512], BF16,
                              kind="ExternalInput").ap()
    d["wvb"] = nc.dram_tensor("wvb", [128, CC, 256], BF16,
                              kind="ExternalInput").ap()
    d["wo"] = nc.dram_tensor("wo", [128, CC, C], BF16,
                             kind="ExternalInput").ap()
    d["w1"] = nc.dram_tensor("w1", [128, 4, CC, C], BF16,
                             kind="ExternalInput").ap()
    d["w2"] = nc.dram_tensor("w2", [128, 4, CC, C], BF16,
                             kind="ExternalInput").ap()
    d["prk"] = nc.dram_tensor("prk", [128, CC, 8], F32, kind="ExternalInput").ap()
    d["b1p"] = nc.dram_tensor("b1p", [128, ICN], F32, kind="ExternalInput").ap()
    d["msk"] = nc.dram_tensor("msk", [128, 32], BF16, kind="ExternalInput").ap()
    d["ones"] = nc.dram_tensor("ones", [128, H], F32R, kind="ExternalInput").ap()
    d["lnz"] = nc.dram_tensor("lnz", [2, 65, TQ], F32R, kind="ExternalInput").ap()
    d["selm"] = nc.dram_tensor("selm", [65, 3, DH], F32R, kind="ExternalInput").ap()
    d["lnt"] = nc.dram_tensor("lnt", [2, 65, C], F32R, kind="ExternalInput").ap()
    d["outT"] = nc.dram_tensor("outT", [C, TQ], F32, kind="ExternalOutput").ap()

    with tile.TileContext(nc) as tc:
        _emit(nc, tc, d)
    nc.finalize()
    return nc


def _ln_bcast(nc, pool, eps_sb, lnr1, lnr2, mean, ex2, tagp):
    """Fill lnr1 (row0 = istd) and lnr2 (row0 = mean*istd, row32 = ones)
    so the LN broadcasts run as K=65 selector matmuls at full PE rate.
    istd = exp(-0.5*ln(var+eps)) on ACT."""
    n = float(C)
    m2 = pool.tile([1, TQ], F32, name="ln_m2", tag=tagp + "m2")
    nc.vector.tensor_mul(m2[:], mean[:], mean[:])
    dv = pool.tile([1, TQ], F32, name="ln_d", tag=tagp + "d")
    nc.vector.tensor_sub(dv[:], ex2[:], m2[:])
    lnv = pool.tile([1, TQ], F32, name="ln_lnv", tag=tagp + "lnv")
    nc.scalar.activation(lnv[:], dv[:], ACTF.Ln,
                         scale=n / (n - 1.0), bias=eps_sb[:])
    with nc.allow_low_precision(reason="f32r matmul operand"):
        nc.scalar.activation(lnr1[0:1, :], lnv[:], ACTF.Exp, scale=-0.5)
        nc.vector.tensor_mul(lnr2[0:1, :], mean[:], lnr1[0:1, :])


def _emit(nc, tc, d):
    # ---- persistent constants ------------------------------------------
    const = tc.alloc_tile_pool(name="const", bufs=1, side="left")
    onesh_sb = const.tile([128, H], F32R, name="onesh_sb")
    selm_sb = const.tile([65, 3, DH], F32R, name="selm_sb")
    lnt1_sb = const.tile([65, C], F32R, name="lnt1_sb")
    lnt2_sb = const.tile([65, C], F32R, name="lnt2_sb")
    eps_sb = const.tile([1, 1], F32, name="eps_sb")
    prk_sb = const.tile([128, CC, 8], F32, name="prk_sb")
    b1p_sb = const.tile([128, ICN], F32, name="b1p_sb")
    mskb_sb = const.tile([128, 32], BF16, name="mskb_sb")

    def prm(cc, pi):
        return prk_sb[:, cc, pi].unsqueeze(-1)  # [128,1]

    nc.vector.memset(eps_sb[:], float(EPS))
    nc.gpsimd.dma_start(out=onesh_sb[:], in_=d["ones"][:])
    ones1_sb = onesh_sb[:, 0:1]
    nc.gpsimd.dma_start(out=prk_sb[:], in_=d["prk"][:])
    nc.gpsimd.dma_start(out=mskb_sb[:], in_=d["msk"][:])
    nc.gpsimd.dma_start(out=selm_sb[:], in_=d["selm"][:])
    nc.gpsimd.dma_start(out=lnt1_sb[:], in_=d["lnt"][0, :, :])
    nc.gpsimd.dma_start(out=lnt2_sb[:], in_=d["lnt"][1, :, :])
    nc.gpsimd.dma_start(out=b1p_sb[:], in_=d["b1p"][:])

    # ---- persistent activation tensors ---------------------------------
    # left-side stack order = reverse release order: ctxT/ctxa released
    # late, qT/kv right after A'
    ctxT_pool = tc.alloc_tile_pool(name="ctxTp", bufs=1, side="left")
    ctxT_sb = [ctxT_pool.tile([128, TQ], BF16, name=f"ctxT{cc}")
               for cc in range(CC)]
    ctxa_pool = tc.alloc_tile_pool(name="ctxap", bufs=1, side="left")
    ctxa_sb = [ctxa_pool.tile([DH + 1, TQ], F32, name=f"ctxa{h}")
               for h in range(H)]
    qT_pool = tc.alloc_tile_pool(name="qTp", bufs=1, side="left")
    qz_sb = [qT_pool.tile([128, TQ], BF16, name=f"qz{h}") for h in range(H)]
    for h in range(H):
        ro = (h % 2) * DH
        with nc.allow_low_precision(reason="zero fill"):
            nc.vector.memset(qz_sb[h][(DH - ro):(128 - ro), :], 0.0)

    kv_pool = tc.alloc_tile_pool(name="kvp", bufs=1, side="left")
    kT_sb = [kv_pool.tile([128, T], BF16, name=f"kT{cc}") for cc in range(CC)]
    v_sb = [kv_pool.tile([128, H, DH + 1], BF16, name=f"v{tch}")
            for tch in range(KCN)]
    for tch in range(KCN):
        with nc.allow_low_precision(reason="ones fill"):
            nc.vector.tensor_copy(v_sb[tch][:, :, DH], onesh_sb[:])

    # ==================== phase A': QKV + scores + exp + ctx ============
    wo_pool = tc.alloc_tile_pool(name="wop", bufs=1, side="right")
    with tc.tile_pool(name="xqs", bufs=1, side="right") as xq_pool, \
         tc.tile_pool(name="wqs", bufs=1, side="right") as wq_pool, \
         tc.tile_pool(name="wkvs", bufs=1, side="right") as wkv_pool, \
         tc.tile_pool(name="xts", bufs=2, side="right") as xt_pool, \
         tc.tile_pool(name="etp", bufs=1, side="right") as et_pool:

        # -- DMA schedule ------------------------------------------------
        # sync q: xtqh chunks, xt blocks, wva/wvb
        # scalar q: wq chunks, wk mc-slabs
        # gpsimd q: small consts (above), wo
        xtqh_pk = xq_pool.tile([128, CC, TQ], BF16, name="xtqh_pk")
        wq_pk = wq_pool.tile([128, CC, C], BF16, name="wq_pk")
        for kc in range(CC):
            nc.sync.dma_start(out=xtqh_pk[:, kc, :], in_=d["xtqh"][:, kc, :])
            nc.scalar.dma_start(out=wq_pk[:, kc, :], in_=d["wq"][:, kc, :])
        wk_pk = wkv_pool.tile([128, CC, CC, 128], BF16, name="wk_pk")
        for mc in range(CC):
            nc.gpsimd.dma_start(out=wk_pk[:, mc, :, :],
                                in_=d["wk"][:, mc, :, :])
        xt_tiles = {}
        for tb in range(2):
            t = xt_pool.tile([128, CC, 512], BF16, name="xt_t", tag="xt")
            nc.sync.dma_start(out=t[:], in_=d["xt"][:, tb, :, :])
            xt_tiles[tb] = t
        wva_pk = wkv_pool.tile([128, CC, 512], BF16, name="wva_pk")
        nc.gpsimd.dma_start(out=wva_pk[:], in_=d["wva"][:])
        wvb_pk = wkv_pool.tile([128, CC, 256], BF16, name="wvb_pk")
        nc.gpsimd.dma_start(out=wvb_pk[:], in_=d["wvb"][:])
        wo_pk = wo_pool.tile([128, CC, C], BF16, name="wo_pk")
        nc.gpsimd.dma_start(out=wo_pk[:], in_=d["wo"][:])
        wo_sb = [wo_pk[:, cc, :] for cc in range(CC)]

        # -- q: kc-outer so first matmul needs only chunk 0 of wq/xtqh --
        with tc.tile_pool(name="pq", bufs=1, space="PSUM") as pq:
            ps_q = [pq.tile([128, TQ], F32, name=f"psq{mc}", tag=f"q{mc}")
                    for mc in range(CC)]
            for kc in range(CC):
                for mc in range(CC):
                    nc.tensor.matmul(ps_q[mc][:],
                                     wq_pk[:, kc, mc * 128:(mc + 1) * 128],
                                     xtqh_pk[:, kc, :],
                                     start=(kc == 0), stop=(kc == CC - 1))
            for mc in range(CC):
                with nc.allow_low_precision(reason="bf16 activations"):
                    nc.vector.tensor_scalar_add(
                        qz_sb[2 * mc][0:DH, :], ps_q[mc][0:DH, :],
                        prm(mc, P_BQ)[0:DH, :])
                    nc.vector.tensor_scalar_add(
                        qz_sb[2 * mc + 1][DH:128, :], ps_q[mc][DH:128, :],
                        prm(mc, P_BQ)[DH:128, :])

        with tc.tile_pool(name="pkv", bufs=1, space="PSUM") as pkv, \
             tc.tile_pool(name="psc", bufs=1, space="PSUM") as psc, \
             tc.tile_pool(name="pctx", bufs=1, space="PSUM") as pctx:

            def emit_k(tb, mc):
                xt_blk = xt_tiles[tb]
                ps = pkv.tile([128, 512], F32, name="ps_k", tag="kv1",
                              bufs=2)
                for kc in range(CC):
                    nc.tensor.matmul(
                        ps[:], wk_pk[:, mc, kc, :], xt_blk[:, kc, :],
                        start=(kc == 0), stop=(kc == CC - 1))
                with nc.allow_low_precision(reason="bf16 activations"):
                    nc.vector.tensor_scalar_add(
                        kT_sb[mc][:, tb * 512:(tb + 1) * 512], ps[:],
                        prm(mc, P_BK))

            def emit_v(tb, tci):
                xt_blk = xt_tiles[tb]
                tch = tb * 4 + tci
                ps1 = pkv.tile([128, 512], F32, name="ps_v1", tag="kv1",
                               bufs=2)
                ps2 = pkv.tile([128, 256], F32, name="ps_v2", tag="v2",
                               bufs=1)
                for kc in range(CC):
                    xsl = xt_blk[:, kc, tci * 128:(tci + 1) * 128]
                    nc.tensor.matmul(ps1[:], xsl, wva_pk[:, kc, :],
                                     start=(kc == 0), stop=(kc == CC - 1))
                    nc.tensor.matmul(ps2[:], xsl, wvb_pk[:, kc, :],
                                     start=(kc == 0), stop=(kc == CC - 1))
                vt = v_sb[tch]
                with nc.allow_low_precision(reason="bf16 activations"):
                    nc.vector.tensor_copy(
                        vt[:, 0:8, 0:DH],
                        ps1[:].rearrange("p (h d) -> p h d", d=DH))
                    nc.vector.tensor_copy(
                        vt[:, 8:H, 0:DH],
                        ps2[:].rearrange("p (h d) -> p h d", d=DH))

            cur_et = {}   # h -> list of (et_tile, off, w, kc) for this tb

            def emit_scores(tb, h):
                cc = h // 2
                kTh = kT_sb[cc]
                qzh = qz_sb[h]
                lst = []
                for kcs in SPACK[tb]:
                    tw = kcs[-1][1] + TQ - 32 * kcs[-1][0]
                    ps = psc.tile([128, 1024], F32, name="ps_s",
                                  tag="sA", bufs=2)
                    for kc, off in kcs:
                        nc.tensor.matmul(
                            ps[:, off:off + TQ - 32 * kc],
                            kTh[:, kc * 128:(kc + 1) * 128],
                            qzh[:, 32 * kc:TQ],
                            start=True, stop=True)
                    et = et_pool.tile([128, 1024], BF16, name="et",
                                      tag="et", bufs=6)
                    nc.scalar.activation(et[:, 0:tw], ps[:, 0:tw], ACTF.Exp,
                                         scale=float(SCALE))
                    for kc, o in kcs:
                        with nc.allow_low_precision(reason="bf16 mask"):
                            nc.vector.tensor_mul(
                                et[:, o:o + 32], et[:, o:o + 32],
                                mskb_sb[:])
                        lst.append((et, o, TQ - 32 * kc, kc))
                cur_et[h] = lst

            def emit_ctx(tb, h):
                ps = pctx.tile([DH + 1, TQ], F32, name="ctx_ps", tag="ctx",
                               bufs=1)
                lst = cur_et.pop(h)
                w0 = lst[0][2]
                for i, (et, o, w, kc) in enumerate(lst):
                    nc.tensor.matmul(ps[:, TQ - w:], v_sb[kc][:, h, :],
                                     et[:, o:o + w],
                                     start=(i == 0), stop=(i == len(lst) - 1))
                if tb == 0:
                    nc.vector.tensor_copy(ctxa_sb[h][:], ps[:])
                else:
                    nc.vector.tensor_add(ctxa_sb[h][:, TQ - w0:],
                                         ctxa_sb[h][:, TQ - w0:],
                                         ps[:, TQ - w0:])

            # tb0 k/v (nothing to interleave yet)
            for mc in range(CC):
                emit_k(0, mc)
            for tci in range(4):
                emit_v(0, tci)

            # windows: scores/exp/ctx of tb interleaved with k/v of tb+1
            for stb in range(TBN):
                ftb = stb + 1
                fills = []
                if ftb < TBN:
                    if ftb + 1 < TBN and (ftb + 1) not in xt_tiles:
                        t2 = xt_pool.tile([128, CC, 512], BF16,
                                          name="xt_t", tag="xt")
                        nc.sync.dma_start(out=t2[:],
                                          in_=d["xt"][:, ftb + 1, :, :])
                        xt_tiles[ftb + 1] = t2
                        del t2
                    fills += [(emit_k, (ftb, mc)) for mc in range(CC)]
                    fills += [(emit_v, (ftb, tci)) for tci in range(4)]
                for h in range(H):
                    emit_scores(stb, h)
                    if h > 0:
                        emit_ctx(stb, h - 1)
                    if h < len(fills):
                        fn, args = fills[h]
                        fn(*args)
                emit_ctx(stb, H - 1)

    kv_pool.release()
    qT_pool.release()

    # ==================== phase B': norm + Wo + LN1 =====================
    w1pool = tc.alloc_tile_pool(name="w1pool", bufs=2, side="right")
    w2pool = tc.alloc_tile_pool(name="w2pool", bufs=2, side="right")
    xtq_pool = tc.alloc_tile_pool(name="xtqp", bufs=1, side="right")

    hT_holder = {}
    with tc.tile_pool(name="bpool", bufs=2, side="right") as bpool, \
         tc.tile_pool(name="r1pool", bufs=1, side="right") as r1pool:

        # streams that waited on qz/kT/v SBUF space
        xtq_pk = xtq_pool.tile([128, CC, TQ], F32R, name="xtq_pk")
        nc.gpsimd.dma_start(out=xtq_pk[:], in_=d["xtq"][:])
        xtq_sb = [xtq_pk[:, cc, :] for cc in range(CC)]
        w1blk = {}
        for jb in range(2):
            t = w1pool.tile([128, CC, C], BF16, name="w1_t", tag="w1")
            nc.gpsimd.dma_start(out=t[:], in_=d["w1"][:, jb, :, :])
            w1blk[jb] = [t[:, kc, :] for kc in range(CC)]
        w2g = {}
        for gg in range(2):
            t = w2pool.tile([128, CC, C], BF16, name="w2_t", tag="w2")
            nc.sync.dma_start(out=t[:], in_=d["w2"][:, gg, :, :])
            w2g[gg] = t

        # softmax normalization: rcp = exp(-ln(dn)) per head on ACT,
        # broadcast via K=65 selector matmul, fused normalize into the
        # ctxa -> ctxT eviction multiply.
        dn3 = bpool.tile([65, TQ], F32R, name="dn3", tag="dn3", bufs=1)
        nc.gpsimd.dma_start(out=dn3[:], in_=d["lnz"][0, :, :])
        with tc.tile_pool(name="ppb", bufs=1, space="PSUM") as ppb:
            for h in range(H):
                cc, ro, j = h // 2, (h % 2) * DH, h % 3
                lntmp = bpool.tile([1, TQ], F32, name="lntmp", tag="lnt")
                nc.scalar.activation(lntmp[:], ctxa_sb[h][DH:DH + 1, :],
                                     ACTF.Ln)
                with nc.allow_low_precision(reason="f32r matmul operand"):
                    nc.scalar.activation(dn3[32 * j:32 * j + 1, :],
                                         lntmp[:], ACTF.Exp, scale=-1.0)
                pb = ppb.tile([DH, TQ], F32, name="pb", tag="pb", bufs=2)
                nc.tensor.matmul(pb[:], selm_sb[:, j, :], dn3[:],
                                 start=True, stop=True)
                with nc.allow_low_precision(reason="bf16 activations"):
                    nc.vector.tensor_mul(ctxT_sb[cc][ro:ro + DH, :],
                                         ctxa_sb[h][0:DH, :], pb[:])

        # Wo + residual + LN1 stats
        r1_sb = [r1pool.tile([128, TQ], F32R, name=f"r1{cc}")
                 for cc in range(CC)]
        lnr1 = bpool.tile([65, TQ], F32R, name="lnr1", tag="lnr1")
        lnr2 = bpool.tile([65, TQ], F32R, name="lnr2", tag="lnr2")
        nc.gpsimd.dma_start(out=lnr1[:], in_=d["lnz"][0, :, :])
        nc.gpsimd.dma_start(out=lnr2[:], in_=d["lnz"][1, :, :])
        with tc.tile_pool(name="pao", bufs=2, space="PSUM") as pao, \
             tc.tile_pool(name="pst", bufs=2, space="PSUM") as pst:
            ps_sum = pst.tile([1, TQ], F32, name="ps_sum", tag="st")
            ps_sq = pst.tile([1, TQ], F32, name="ps_sq", tag="st")
            for mc in range(CC):
                ps = pao.tile([128, TQ], F32, name="ps_ao", tag="ao")
                for kc in range(CC):
                    nc.tensor.matmul(ps[:],
                                     wo_sb[kc][:, mc * 128:(mc + 1) * 128],
                                     ctxT_sb[kc][:],
                                     start=(kc == 0), stop=(kc == CC - 1))
                nc.vector.scalar_tensor_tensor(
                    r1_sb[mc][:], ps[:], prm(mc, P_BO), xtq_sb[mc][:],
                    mybir.AluOpType.add, mybir.AluOpType.add)
                nc.tensor.matmul(ps_sum[:], ones1_sb[:], r1_sb[mc][:],
                                 start=(mc == 0), stop=(mc == CC - 1))
                sq = bpool.tile([128, TQ], F32R, name="sq", tag="sq")
                nc.scalar.activation(sq[:], r1_sb[mc][:], ACTF.Square)
                nc.tensor.matmul(ps_sq[:], ones1_sb[:], sq[:],
                                 start=(mc == 0), stop=(mc == CC - 1))
            n = float(C)
            mean1 = bpool.tile([1, TQ], F32R, name="l1mean", tag="l1mean")
            nc.scalar.activation(mean1[:], ps_sum[:], ACTF.Copy, scale=1.0 / n)
            ex21 = bpool.tile([1, TQ], F32, name="l1ex2", tag="l1ex2")
            nc.scalar.activation(ex21[:], ps_sq[:], ACTF.Copy, scale=1.0 / n)
        hT_pool = tc.alloc_tile_pool(name="hTp", bufs=1, side="left")
        hT_sb = [hT_pool.tile([128, TQ], BF16, name=f"hT{cc}")
                 for cc in range(CC)]
        hT_holder["pool"] = hT_pool
        _ln_bcast(nc, bpool, eps_sb, lnr1, lnr2, mean1, ex21, "l1")
        with tc.tile_pool(name="pbc2", bufs=2, space="PSUM") as pbc2:
            for cc in range(CC):
                csl = slice(cc * 128, (cc + 1) * 128)
                pb2 = pbc2.tile([128, 2 * TQ], F32, name="lnpb", tag="bc")
                nc.tensor.matmul(pb2[:, 0:TQ], lnt1_sb[:, csl],
                                 lnr1[:], start=True, stop=True)
                nc.tensor.matmul(pb2[:, TQ:], lnt1_sb[:, csl],
                                 lnr2[:], start=True, stop=True)
                t1 = bpool.tile([128, TQ], F32, name="ln_t1", tag="lnt1")
                nc.vector.tensor_mul(t1[:], r1_sb[cc][:], pb2[:, 0:TQ])
                with nc.allow_low_precision(reason="bf16 activations"):
                    nc.vector.tensor_sub(hT_sb[cc][:], t1[:], pb2[:, TQ:])

    xtq_pool.release()

    # ==================== phase D: MLP + residual + LN2 =================
    with tc.tile_pool(name="dpool", bufs=2, side="right") as dpool, \
         tc.tile_pool(name="r2pool", bufs=1, side="right") as r2pool:

        r2_sb = [r2pool.tile([128, TQ], F32R, name=f"r2{cc}")
                 for cc in range(CC)]
        lnr1b = dpool.tile([65, TQ], F32R, name="lnr1b", tag="lnr1b")
        lnr2b = dpool.tile([65, TQ], F32R, name="lnr2b", tag="lnr2b")
        nc.gpsimd.dma_start(out=lnr1b[:], in_=d["lnz"][0, :, :])
        nc.gpsimd.dma_start(out=lnr2b[:], in_=d["lnz"][1, :, :])
        with tc.tile_pool(name="pfc2", bufs=1, space="PSUM") as pfc2:
            ps_m = [pfc2.tile([128, TQ], F32, name=f"ps_m{mc}", tag=f"m{mc}")
                    for mc in range(CC)]
            with tc.tile_pool(name="pfc1", bufs=2, space="PSUM") as pfc1:
                for kc2 in range(ICN):
                    jb = kc2 // CC
                    ps1 = pfc1.tile([128, TQ], F32, name="ps1", tag="f1")
                    co = (kc2 % CC) * 128
                    for kc in range(CC):
                        nc.tensor.matmul(
                            ps1[:], w1blk[jb][kc][:, co:co + 128],
                            hT_sb[kc][:],
                            start=(kc == 0), stop=(kc == CC - 1))
                    g = dpool.tile([128, TQ], BF16, name="g", tag="g")
                    with nc.allow_low_precision(reason="bf16 activations"):
                        nc.scalar.activation(g[:], ps1[:],
                                             ACTF.Gelu_apprx_tanh,
                                             bias=b1p_sb[:, kc2].unsqueeze(-1))
                    w2t = w2g[kc2 // CC][:, kc2 % CC, :]
                    for mc in range(CC):
                        nc.tensor.matmul(ps_m[mc][:],
                                         w2t[:, mc * 128:(mc + 1) * 128],
                                         g[:], start=(kc2 == 0),
                                         stop=(kc2 == ICN - 1))
                    # ring prefetches: issued after this iteration's readers
                    if kc2 % CC == CC - 1 and jb + 2 <= 3:
                        t = w1pool.tile([128, CC, C], BF16, name="w1_t",
                                        tag="w1")
                        nc.gpsimd.dma_start(out=t[:],
                                            in_=d["w1"][:, jb + 2, :, :])
                        w1blk[jb + 2] = [t[:, kc, :] for kc in range(CC)]
                        t2 = w2pool.tile([128, CC, C], BF16, name="w2_t",
                                         tag="w2")
                        nc.sync.dma_start(out=t2[:],
                                          in_=d["w2"][:, jb + 2, :, :])
                        w2g[jb + 2] = t2
            with tc.tile_pool(name="pst2", bufs=2, space="PSUM") as pst2:
                ps_sum2 = pst2.tile([1, TQ], F32, name="ps_sum2", tag="st")
                ps_sq2 = pst2.tile([1, TQ], F32, name="ps_sq2", tag="st")
                for mc in range(CC):
                    nc.vector.scalar_tensor_tensor(
                        r2_sb[mc][:], ps_m[mc][:], prm(mc, P_B2),
                        hT_sb[mc][:], mybir.AluOpType.add,
                        mybir.AluOpType.add)
                    nc.tensor.matmul(ps_sum2[:], ones1_sb[:], r2_sb[mc][:],
                                     start=(mc == 0), stop=(mc == CC - 1))
                    sq = dpool.tile([128, TQ], F32R, name="sq2", tag="sq")
                    nc.scalar.activation(sq[:], r2_sb[mc][:], ACTF.Square)
                    nc.tensor.matmul(ps_sq2[:], ones1_sb[:], sq[:],
                                     start=(mc == 0), stop=(mc == CC - 1))
                n = float(C)
                mean2 = dpool.tile([1, TQ], F32R, name="l2mean", tag="l2mean")
                nc.scalar.activation(mean2[:], ps_sum2[:], ACTF.Copy,
                                     scale=1.0 / n)
                ex22 = dpool.tile([1, TQ], F32, name="l2ex2", tag="l2ex2")
                nc.scalar.activation(ex22[:], ps_sq2[:], ACTF.Copy,
                                     scale=1.0 / n)
        hT_holder["pool"].release()
        ctxa_pool.release()
        _ln_bcast(nc, dpool, eps_sb, lnr1b, lnr2b, mean2, ex22, "l2")
        with tc.tile_pool(name="pbc3", bufs=2, space="PSUM") as pbc3:
            for cc in range(CC):
                csl = slice(cc * 128, (cc + 1) * 128)
                pb3 = pbc3.tile([128, 2 * TQ], F32, name="lnpb3", tag="bc")
                nc.tensor.matmul(pb3[:, 0:TQ], lnt2_sb[:, csl],
                                 lnr1b[:], start=True, stop=True)
                nc.tensor.matmul(pb3[:, TQ:], lnt2_sb[:, csl],
                                 lnr2b[:], start=True, stop=True)
                t1 = dpool.tile([128, TQ], F32, name="ln_t13", tag="lnt13")
                nc.vector.tensor_mul(t1[:], r2_sb[cc][:], pb3[:, 0:TQ])
                ot = dpool.tile([128, TQ], F32, name=f"o{cc}", tag=f"o{cc}",
                                bufs=1)
                nc.vector.tensor_sub(ot[:], t1[:], pb3[:, TQ:])
                nc.sync.dma_start(out=d["outT"][cc * 128:(cc + 1) * 128, :],
                                  in_=ot[:])

    w2pool.release()
    w1pool.release()
    wo_pool.release()
    ctxT_pool.release()
    const.release()


_NC = None


def _get_nc():
    global _NC
    if _NC is None:
        _NC = _build_nc()
    return _NC


def _prep_inmaps(x, Wq, bq, Wk, bk, Wv, bv, Wo, bo, ln1_s, ln1_b,
                 W1, b1, W2, b2, ln2_s, ln2_b):
    f32 = np.float32
    bf16 = ml_dtypes.bfloat16

    def pk(a):
        # [A*128, c...] -> [128, A, c...] contiguous (partition-major)
        a = np.asarray(a)
        return np.ascontiguousarray(
            a.reshape(-1, 128, *a.shape[1:]).swapaxes(0, 1))

    xT = [np.ascontiguousarray(np.asarray(x)[b].T, dtype=f32)
          for b in range(B)]
    xTh = [xb.astype(bf16) for xb in xT]
    wq = pk(np.asarray(Wq, f32).astype(bf16))
    # wk packed per-mc-slab: [128, mc, kc, 128]
    wk_p = pk(np.asarray(Wk, f32).astype(bf16))        # [128, kc, 768]
    wk = np.ascontiguousarray(
        wk_p.reshape(128, CC, CC, 128).swapaxes(1, 2))
    wv_p = pk(np.asarray(Wv, f32).astype(bf16))
    wva = np.ascontiguousarray(wv_p[:, :, 0:512])
    wvb = np.ascontiguousarray(wv_p[:, :, 512:768])
    wo = pk(np.asarray(Wo, f32).astype(bf16))
    w1f = np.asarray(W1, f32).astype(bf16)
    w1 = np.stack([pk(w1f[:, jb * C:(jb + 1) * C]) for jb in range(4)],
                  axis=1)
    w2f = np.asarray(W2, f32).astype(bf16)
    w2 = np.stack([pk(w2f[gg * C:(gg + 1) * C, :]) for gg in range(4)],
                  axis=1)
    # fold bv into bo: ctx_norm = ctx*rcp absorbs +bv exactly
    bo_f = np.asarray(bo, f32) + np.asarray(bv, f32) @ np.asarray(Wo, f32)
    prk = np.zeros((128, CC, 8), f32)
    for pi, arr in ((P_BQ, bq), (P_BK, bk), (P_BO, bo_f), (P_B2, b2),
                    (P_L1S, ln1_s), (P_L1B, ln1_b), (P_L2S, ln2_s),
                    (P_L2B, ln2_b)):
        prk[:, :, pi] = np.asarray(arr, f32).reshape(CC, 128).T
    b1p = np.ascontiguousarray(np.asarray(b1, f32).reshape(ICN, 128).T)
    lnt = np.zeros((2, 65, C), f32)
    lnt[0, 0], lnt[0, 32] = np.asarray(ln1_s, f32), -np.asarray(ln1_b, f32)
    lnt[1, 0], lnt[1, 32] = np.asarray(ln2_s, f32), -np.asarray(ln2_b, f32)
    selm = np.zeros((65, 3, DH), f32)
    for j in range(3):
        selm[32 * j, j, :] = 1.0
    ones = np.ones((128, H), f32)
    lnz = np.zeros((2, 65, TQ), f32)
    lnz[1, 32, :] = 1.0

    kk = np.arange(128)[:, None]
    in_maps = []
    for c in range(8):
        b, p = c // 4, c % 4
        qq = np.arange(32)[None, :]
        msk = np.ascontiguousarray(
            (kk <= 4 * qq + p).astype(bf16))  # k <= 4j+p, all chunks

        xtp = pk(xTh[b])  # [128, CC, T]
        xtp = np.ascontiguousarray(
            xtp.reshape(128, CC, TBN, 512).swapaxes(1, 2))
        in_maps.append({
            "xt": xtp,
            "xtqh": pk(np.ascontiguousarray(xTh[b][:, p::4])),
            "xtq": pk(np.ascontiguousarray(xT[b][:, p::4])),
            "wq": wq, "wk": wk, "wva": wva, "wvb": wvb, "wo": wo,
            "w1": w1, "w2": w2,
            "prk": prk, "b1p": b1p, "msk": msk,
            "lnt": lnt, "selm": selm, "ones": ones, "lnz": lnz,
        })
    return in_maps


def _run(in_maps, trace=False, **kw):
    nc = _get_nc()
    return run_bass_kernel_spmd(nc, in_maps, list(range(8)), trace=trace, **kw)


def kernel(**inputs):
    in_maps = _prep_inmaps(**inputs)
    res = _run(in_maps)
    out = np.empty((B, T, C), np.float32)
    for c in range(8):
        b, p = c // 4, c % 4
        out[b, p::4, :] = res.results[c]["outT"].T
    return out


# revision 32
# speedup vs baseline: 1.0896x; 1.0698x over previous
"""Trainium2 Bass kernel for one transformer block (B=2, T=2048, C=768, H=12,
inner=3072, fp32 I/O, causal, post-norm residual).

Sharding: 8 cores, token-interleaved. Core c handles batch c//4, tokens
p::4 (p = c%4) of that batch — every core runs the IDENTICAL program
(SPMD); causality is data-driven via per-core mask tensors.

v3 (vs 292us baseline): fused A' pipeline — scores+exp+ctx interleaved
INTO the K/V token-block loop so the ACT engine's ~60us of exp work (the
old phase-B bottleneck) hides under QKV matmuls:
- per tb: k, v computed; scores for the PREVIOUS tb's chunks run
  interleaved per head (score tiles -> exp -> boundary masks on the Pool
  engine -> ctx partial matmuls -> DVE accumulate into per-head f32
  SBUF accumulators ctxa [65,TQ]). PSUM: kv ring2 (2 banks) + v-ps2 (1)
  + score [128,1024] ring2 (4) + ctx [65,512] ring1 (1) = 8 exactly.
- q is kc-outer (6 psum banks, before the tb loop) with per-chunk
  wq/xtqh DMAs so the first matmul fires after ~330KB lands (was ~2MB).
- bv folded host-side into bo' = bo + bv @ Wo (ctx_norm = ctx*rcp
  absorbs +bv exactly via the denominator row): v eviction is a pure
  copy, no bias tile.
- softmax reciprocal = exp(-ln(dn)) on ACT (idle in B'), per head; the
  old 14.6us DVE reciprocal is gone. K=65 selector matmul broadcasts
  rcp; the psum->sbuf ctx eviction fuses into the normalize multiply.
- wk packed per-mc-slab so k(mc) streams as slabs land; wv split
  512/256; w1/w2 + xtq stream during B' when qz/kT/v free up.
"""

import sys

if "/opt/trn_rl_repo" not in sys.path:
    sys.path.insert(0, "/opt/trn_rl_repo")

import numpy as np
import ml_dtypes

import concourse.bacc as bacc
import concourse.mybir as mybir
import concourse.tile as tile
from concourse.bass_utils import run_bass_kernel_spmd

F32 = mybir.dt.float32
F32R = mybir.dt.float32r
BF16 = mybir.dt.bfloat16
ACTF = mybir.ActivationFunctionType

B, T, C = 2, 2048, 768
H, DH = 12, 64
IN = 3072
CC = C // 128          # 6 channel chunks
TBN = T // 512         # 4 token blocks of full seq
TQ = 512               # tokens per core
KCN = T // 128         # 16 k-chunks
ICN = IN // 128        # 24 inner chunks
EPS = 1e-4
SCALE = 1.0 / np.sqrt(DH)

# param pack order in "prk" [128, CC, 8]
P_BQ, P_BK, P_BO, P_B2, P_L1S, P_L1B, P_L2S, P_L2B = range(8)

# score-tile packing per tb: tiles of (global kc chunk, col offset).
# Chunk kc covers queries 32*kc..TQ (width TQ-32*kc). Offsets chosen so
# NO matmul output crosses a 2KB psum bank boundary (512 f32 cols) —
# a crossing write half-overwrites / half-accumulates-onto-stale.
# Gaps between chunks get exp'd (garbage, never read) — harmless.
SPACK = [
    [[(0, 0), (1, 512)], [(2, 0), (3, 512)]],
    [[(4, 0), (5, 512)], [(6, 0), (7, 512)]],
    [[(8, 0), (9, 256), (10, 512), (11, 704)]],
    [[(12, 0), (13, 128), (14, 224), (15, 288)]],
]


def _build_nc():
    nc = bacc.Bacc("TRN2", target_bir_lowering=False, debug=False,
                   enable_asserts=False, num_devices=8)
    d = {}
    d["xt"] = nc.dram_tensor("xt", [128, TBN, CC, 512], BF16,
                             kind="ExternalInput").ap()
    d["xtqh"] = nc.dram_tensor("xtqh", [128, CC, TQ], BF16,
                               kind="ExternalInput").ap()
    d["xtq"] = nc.dram_tensor("xtq", [128, CC, TQ], F32R,
                              kind="ExternalInput").ap()
    d["wq"] = nc.dram_tensor("wq", [128, CC, C], BF16,
                             kind="ExternalInput").ap()
    d["wk"] = nc.dram_tensor("wk", [128, CC, CC, 128], BF16,
                             kind="ExternalInput").ap()  # [p, mc, kc, j]
    d["wva"] = nc.dram_tensor("wva", [128, CC, 512], BF16,
                              kind="ExternalInput").ap()
    d["wvb"] = nc.dram_tensor("wvb", [128, CC, 256], BF16,
                              kind="ExternalInput").ap()
    d["wo"] = nc.dram_tensor("wo", [128, CC, C], BF16,
                             kind="ExternalInput").ap()
    d["w1"] = nc.dram_tensor("w1", [128, 4, CC, C], BF16,
                             kind="ExternalInput").ap()
    d["w2"] = nc.dram_tensor("w2", [128, 4, CC, C], BF16,
                             kind="ExternalInput").ap()
    d["prk"] = nc.dram_tensor("prk", [128, CC, 8], F32, kind="ExternalInput").ap()
    d["b1p"] = nc.dram_tensor("b1p", [128, ICN], F32, kind="ExternalInput").ap()
    d["msk"] = nc.dram_tensor("msk", [128, 32], BF16, kind="ExternalInput").ap()
    d["ones"] = nc.dram_tensor("ones", [128, H], F32R, kind="ExternalInput").ap()
    d["lnz"] = nc.dram_tensor("lnz", [2, 65, TQ], F32R, kind="ExternalInput").ap()
    d["selm"] = nc.dram_tensor("selm", [65, 3, DH], F32R, kind="ExternalInput").ap()
    d["lnt"] = nc.dram_tensor("lnt", [2, 65, C], F32R, kind="ExternalInput").ap()
    d["outT"] = nc.dram_tensor("outT", [C, TQ], F32, kind="ExternalOutput").ap()

    with tile.TileContext(nc) as tc:
        _emit(nc, tc, d)
    nc.finalize()
    return nc


def _ln_bcast(nc, pool, eps_sb, lnr1, lnr2, mean, ex2, tagp):
    """Fill lnr1 (row0 = istd) and lnr2 (row0 = mean*istd, row32 = ones)
    so the LN broadcasts run as K=65 selector matmuls at full PE rate.
    istd = exp(-0.5*ln(var+eps)) on ACT."""
    n = float(C)
    m2 = pool.tile([1, TQ], F32, name="ln_m2", tag=tagp + "m2")
    nc.vector.tensor_mul(m2[:], mean[:], mean[:])
    dv = pool.tile([1, TQ], F32, name="ln_d", tag=tagp + "d")
    nc.vector.tensor_sub(dv[:], ex2[:], m2[:])
    lnv = pool.tile([1, TQ], F32, name="ln_lnv", tag=tagp + "lnv")
    nc.scalar.activation(lnv[:], dv[:], ACTF.Ln,
                         scale=n / (n - 1.0), bias=eps_sb[:])
    with nc.allow_low_precision(reason="f32r matmul operand"):
        nc.scalar.activation(lnr1[0:1, :], lnv[:], ACTF.Exp, scale=-0.5)
        nc.vector.tensor_mul(lnr2[0:1, :], mean[:], lnr1[0:1, :])


def _emit(nc, tc, d):
    # ---- persistent constants ------------------------------------------
    const = tc.alloc_tile_pool(name="const", bufs=1, side="left")
    onesh_sb = const.tile([128, H], F32R, name="onesh_sb")
    selm_sb = const.tile([65, 3, DH], F32R, name="selm_sb")
    lnt1_sb = const.tile([65, C], F32R, name="lnt1_sb")
    lnt2_sb = const.tile([65, C], F32R, name="lnt2_sb")
    eps_sb = const.tile([1, 1], F32, name="eps_sb")
    prk_sb = const.tile([128, CC, 8], F32, name="prk_sb")
    b1p_sb = const.tile([128, ICN], F32, name="b1p_sb")
    mskb_sb = const.tile([128, 32], BF16, name="mskb_sb")

    def prm(cc, pi):
        return prk_sb[:, cc, pi].unsqueeze(-1)  # [128,1]

    nc.vector.memset(eps_sb[:], float(EPS))
    nc.gpsimd.dma_start(out=onesh_sb[:], in_=d["ones"][:])
    ones1_sb = onesh_sb[:, 0:1]
    nc.gpsimd.dma_start(out=prk_sb[:], in_=d["prk"][:])
    nc.gpsimd.dma_start(out=mskb_sb[:], in_=d["msk"][:])
    nc.gpsimd.dma_start(out=selm_sb[:], in_=d["selm"][:])
    nc.gpsimd.dma_start(out=lnt1_sb[:], in_=d["lnt"][0, :, :])
    nc.gpsimd.dma_start(out=lnt2_sb[:], in_=d["lnt"][1, :, :])
    nc.gpsimd.dma_start(out=b1p_sb[:], in_=d["b1p"][:])

    # ---- persistent activation tensors ---------------------------------
    # left-side stack order = reverse release order: ctxT/ctxa released
    # late, qT/kv right after A'
    ctxT_pool = tc.alloc_tile_pool(name="ctxTp", bufs=1, side="left")
    ctxT_sb = [ctxT_pool.tile([128, TQ], BF16, name=f"ctxT{cc}")
               for cc in range(CC)]
    ctxa_pool = tc.alloc_tile_pool(name="ctxap", bufs=1, side="left")
    ctxa_sb = [ctxa_pool.tile([DH + 1, TQ], F32, name=f"ctxa{h}")
               for h in range(H)]
    qT_pool = tc.alloc_tile_pool(name="qTp", bufs=1, side="left")
    qz_sb = [qT_pool.tile([128, TQ], BF16, name=f"qz{h}") for h in range(H)]
    for h in range(H):
        ro = (h % 2) * DH
        with nc.allow_low_precision(reason="zero fill"):
            nc.vector.memset(qz_sb[h][(DH - ro):(128 - ro), :], 0.0)

    kv_pool = tc.alloc_tile_pool(name="kvp", bufs=1, side="left")
    kT_sb = [kv_pool.tile([128, T], BF16, name=f"kT{cc}") for cc in range(CC)]
    v_sb = [kv_pool.tile([128, H, DH + 1], BF16, name=f"v{tch}")
            for tch in range(KCN)]
    for tch in range(KCN):
        with nc.allow_low_precision(reason="ones fill"):
            nc.vector.tensor_copy(v_sb[tch][:, :, DH], onesh_sb[:])

    # ==================== phase A': QKV + scores + exp + ctx ============
    wo_pool = tc.alloc_tile_pool(name="wop", bufs=1, side="right")
    with tc.tile_pool(name="xqs", bufs=1, side="right") as xq_pool, \
         tc.tile_pool(name="wqs", bufs=1, side="right") as wq_pool, \
         tc.tile_pool(name="wkvs", bufs=1, side="right") as wkv_pool, \
         tc.tile_pool(name="xts", bufs=2, side="right") as xt_pool, \
         tc.tile_pool(name="etp", bufs=1, side="right") as et_pool:

        # -- DMA schedule ------------------------------------------------
        # sync q: xtqh chunks, xt blocks, wva/wvb
        # scalar q: wq chunks, wk mc-slabs
        # gpsimd q: small consts (above), wo
        xtqh_pk = xq_pool.tile([128, CC, TQ], BF16, name="xtqh_pk")
        wq_pk = wq_pool.tile([128, CC, C], BF16, name="wq_pk")
        for kc in range(CC):
            nc.sync.dma_start(out=xtqh_pk[:, kc, :], in_=d["xtqh"][:, kc, :])
            nc.scalar.dma_start(out=wq_pk[:, kc, :], in_=d["wq"][:, kc, :])
        wk_pk = wkv_pool.tile([128, CC, CC, 128], BF16, name="wk_pk")
        for mc in range(CC):
            nc.gpsimd.dma_start(out=wk_pk[:, mc, :, :],
                                in_=d["wk"][:, mc, :, :])
        xt_tiles = {}
        for tb in range(2):
            t = xt_pool.tile([128, CC, 512], BF16, name="xt_t", tag="xt")
            nc.sync.dma_start(out=t[:], in_=d["xt"][:, tb, :, :])
            xt_tiles[tb] = t
        wva_pk = wkv_pool.tile([128, CC, 512], BF16, name="wva_pk")
        nc.gpsimd.dma_start(out=wva_pk[:], in_=d["wva"][:])
        wvb_pk = wkv_pool.tile([128, CC, 256], BF16, name="wvb_pk")
        nc.gpsimd.dma_start(out=wvb_pk[:], in_=d["wvb"][:])
        wo_pk = wo_pool.tile([128, CC, C], BF16, name="wo_pk")
        nc.gpsimd.dma_start(out=wo_pk[:], in_=d["wo"][:])
        wo_sb = [wo_pk[:, cc, :] for cc in range(CC)]

        # -- q: kc-outer so first matmul needs only chunk 0 of wq/xtqh --
        with tc.tile_pool(name="pq", bufs=1, space="PSUM") as pq:
            ps_q = [pq.tile([128, TQ], F32, name=f"psq{mc}", tag=f"q{mc}")
                    for mc in range(CC)]
            for kc in range(CC):
                for mc in range(CC):
                    nc.tensor.matmul(ps_q[mc][:],
                                     wq_pk[:, kc, mc * 128:(mc + 1) * 128],
                                     xtqh_pk[:, kc, :],
                                     start=(kc == 0), stop=(kc == CC - 1))
            for mc in range(CC):
                with nc.allow_low_precision(reason="bf16 activations"):
                    nc.vector.tensor_scalar_add(
                        qz_sb[2 * mc][0:DH, :], ps_q[mc][0:DH, :],
                        prm(mc, P_BQ)[0:DH, :])
                    nc.vector.tensor_scalar_add(
                        qz_sb[2 * mc + 1][DH:128, :], ps_q[mc][DH:128, :],
                        prm(mc, P_BQ)[DH:128, :])

        with tc.tile_pool(name="psc", bufs=1, space="PSUM") as psc, \
             tc.tile_pool(name="pctx", bufs=1, space="PSUM") as pctx:

            cur_et = {}   # h -> list of (et_tile, off, w, kc) for this tb

            def emit_scores(tb, h):
                cc = h // 2
                kTh = kT_sb[cc]
                qzh = qz_sb[h]
                lst = []
                for kcs in SPACK[tb]:
                    tw = kcs[-1][1] + TQ - 32 * kcs[-1][0]
                    ps = psc.tile([128, 1024], F32, name="ps_s",
                                  tag="sA", bufs=2)
                    for kc, off in kcs:
                        nc.tensor.matmul(
                            ps[:, off:off + TQ - 32 * kc],
                            kTh[:, kc * 128:(kc + 1) * 128],
                            qzh[:, 32 * kc:TQ],
                            start=True, stop=True)
                    et = et_pool.tile([128, 1024], BF16, name="et",
                                      tag="et", bufs=6)
                    nc.scalar.activation(et[:, 0:tw], ps[:, 0:tw], ACTF.Exp,
                                         scale=float(SCALE))
                    for kc, o in kcs:
                        with nc.allow_low_precision(reason="bf16 mask"):
                            nc.vector.tensor_mul(
                                et[:, o:o + 32], et[:, o:o + 32],
                                mskb_sb[:])
                        lst.append((et, o, TQ - 32 * kc, kc))
                cur_et[h] = lst

            def emit_ctx(tb, h):
                ps = pctx.tile([DH + 1, TQ], F32, name="ctx_ps", tag="ctx",
                               bufs=1)
                lst = cur_et.pop(h)
                w0 = lst[0][2]
                for i, (et, o, w, kc) in enumerate(lst):
                    nc.tensor.matmul(ps[:, TQ - w:], v_sb[kc][:, h, :],
                                     et[:, o:o + w],
                                     start=(i == 0), stop=(i == len(lst) - 1))
                if tb == 0:
                    nc.vector.tensor_copy(ctxa_sb[h][:], ps[:])
                else:
                    nc.vector.tensor_add(ctxa_sb[h][:, TQ - w0:],
                                         ctxa_sb[h][:, TQ - w0:],
                                         ps[:, TQ - w0:])

            # windows 0-2: scores/exp/ctx of tb interleaved with k/v(tb+1)
            with tc.tile_pool(name="pkv", bufs=1, space="PSUM") as pkv:

                def emit_k(tb, mc):
                    xt_blk = xt_tiles[tb]
                    ps = pkv.tile([128, 512], F32, name="ps_k", tag="kv1",
                                  bufs=2)
                    for kc in range(CC):
                        nc.tensor.matmul(
                            ps[:], wk_pk[:, mc, kc, :], xt_blk[:, kc, :],
                            start=(kc == 0), stop=(kc == CC - 1))
                    with nc.allow_low_precision(reason="bf16 activations"):
                        nc.vector.tensor_scalar_add(
                            kT_sb[mc][:, tb * 512:(tb + 1) * 512], ps[:],
                            prm(mc, P_BK))

                def emit_v(tb, tci):
                    xt_blk = xt_tiles[tb]
                    tch = tb * 4 + tci
                    ps1 = pkv.tile([128, 512], F32, name="ps_v1", tag="kv1",
                                   bufs=2)
                    ps2 = pkv.tile([128, 256], F32, name="ps_v2", tag="v2",
                                   bufs=1)
                    for kc in range(CC):
                        xsl = xt_blk[:, kc, tci * 128:(tci + 1) * 128]
                        nc.tensor.matmul(ps1[:], xsl, wva_pk[:, kc, :],
                                         start=(kc == 0), stop=(kc == CC - 1))
                        nc.tensor.matmul(ps2[:], xsl, wvb_pk[:, kc, :],
                                         start=(kc == 0), stop=(kc == CC - 1))
                    vt = v_sb[tch]
                    with nc.allow_low_precision(reason="bf16 activations"):
                        nc.vector.tensor_copy(
                            vt[:, 0:8, 0:DH],
                            ps1[:].rearrange("p (h d) -> p h d", d=DH))
                        nc.vector.tensor_copy(
                            vt[:, 8:H, 0:DH],
                            ps2[:].rearrange("p (h d) -> p h d", d=DH))

                # tb0 k/v (nothing to interleave yet)
                for mc in range(CC):
                    emit_k(0, mc)
                for tci in range(4):
                    emit_v(0, tci)

                for stb in range(TBN - 1):
                    ftb = stb + 1
                    if ftb + 1 < TBN:
                        t2 = xt_pool.tile([128, CC, 512], BF16,
                                          name="xt_t", tag="xt")
                        nc.sync.dma_start(out=t2[:],
                                          in_=d["xt"][:, ftb + 1, :, :])
                        xt_tiles[ftb + 1] = t2
                        del t2
                    fills = [(emit_k, (ftb, mc)) for mc in range(CC)]
                    fills += [(emit_v, (ftb, tci)) for tci in range(4)]
                    for h in range(H):
                        emit_scores(stb, h)
                        if h > 0:
                            emit_ctx(stb, h - 1)
                        if h < len(fills):
                            fn, args = fills[h]
                            fn(*args)
                    emit_ctx(stb, H - 1)

            # window 3: tb3 scores/ctx with softmax normalization fused in
            # (kv banks freed above; pb broadcasts use them). rcp on DVE —
            # ln/exp tables are in different ACT sets, a per-head ln/exp
            # reciprocal costs 2 table loads (1.28us each).
            with tc.tile_pool(name="ppb", bufs=1, space="PSUM") as ppb:
                dnb = [et_pool.tile([65, TQ], F32, name=f"dnb{i}")
                       for i in range(2)]
                nc.vector.memset(dnb[0][:], 1.0)
                nc.vector.memset(dnb[1][:], 1.0)

                def emit_norm(bi):
                    dnt = dnb[bi % 2]
                    for j in range(3):
                        h = 3 * bi + j
                        nc.vector.tensor_copy(dnt[32 * j:32 * j + 1, :],
                                              ctxa_sb[h][DH:DH + 1, :])
                    rcp = et_pool.tile([65, TQ], F32R, name="rcp",
                                       tag="rcp", bufs=2)
                    for q0 in range(0, TQ, 128):
                        with nc.allow_low_precision(reason="f32r operand"):
                            nc.vector.reciprocal(rcp[:, q0:q0 + 128],
                                                 dnt[:, q0:q0 + 128])
                    for j in range(3):
                        h = 3 * bi + j
                        cc, ro = h // 2, (h % 2) * DH
                        pb = ppb.tile([DH, TQ], F32, name="pb", tag="pb",
                                      bufs=2)
                        nc.tensor.matmul(pb[:], selm_sb[:, j, :], rcp[:],
                                         start=True, stop=True)
                        with nc.allow_low_precision(reason="bf16 out"):
                            nc.vector.tensor_mul(
                                ctxT_sb[cc][ro:ro + DH, :],
                                ctxa_sb[h][0:DH, :], pb[:])

                for h in range(H):
                    emit_scores(3, h)
                    if h > 0:
                        emit_ctx(3, h - 1)
                    if h in (4, 7, 10):
                        emit_norm((h - 4) // 3)
                emit_ctx(3, H - 1)
                emit_norm(3)

    kv_pool.release()
    qT_pool.release()

    # ==================== phase B': norm + Wo + LN1 =====================
    w1pool = tc.alloc_tile_pool(name="w1pool", bufs=2, side="right")
    w2pool = tc.alloc_tile_pool(name="w2pool", bufs=2, side="right")
    xtq_pool = tc.alloc_tile_pool(name="xtqp", bufs=1, side="right")

    hT_holder = {}
    with tc.tile_pool(name="bpool", bufs=2, side="right") as bpool, \
         tc.tile_pool(name="r1pool", bufs=1, side="right") as r1pool:

        # streams that waited on qz/kT/v SBUF space
        xtq_pk = xtq_pool.tile([128, CC, TQ], F32R, name="xtq_pk")
        nc.gpsimd.dma_start(out=xtq_pk[:], in_=d["xtq"][:])
        xtq_sb = [xtq_pk[:, cc, :] for cc in range(CC)]
        w1blk = {}
        for jb in range(2):
            t = w1pool.tile([128, CC, C], BF16, name="w1_t", tag="w1")
            nc.gpsimd.dma_start(out=t[:], in_=d["w1"][:, jb, :, :])
            w1blk[jb] = [t[:, kc, :] for kc in range(CC)]
        w2g = {}
        for gg in range(2):
            t = w2pool.tile([128, CC, C], BF16, name="w2_t", tag="w2")
            nc.sync.dma_start(out=t[:], in_=d["w2"][:, gg, :, :])
            w2g[gg] = t

        # Wo + residual + LN1 stats
        r1_sb = [r1pool.tile([128, TQ], F32R, name=f"r1{cc}")
                 for cc in range(CC)]
        lnr1 = bpool.tile([65, TQ], F32R, name="lnr1", tag="lnr1")
        lnr2 = bpool.tile([65, TQ], F32R, name="lnr2", tag="lnr2")
        nc.gpsimd.dma_start(out=lnr1[:], in_=d["lnz"][0, :, :])
        nc.gpsimd.dma_start(out=lnr2[:], in_=d["lnz"][1, :, :])
        with tc.tile_pool(name="pao", bufs=2, space="PSUM") as pao, \
             tc.tile_pool(name="pst", bufs=2, space="PSUM") as pst:
            ps_sum = pst.tile([1, TQ], F32, name="ps_sum", tag="st")
            ps_sq = pst.tile([1, TQ], F32, name="ps_sq", tag="st")
            for mc in range(CC):
                ps = pao.tile([128, TQ], F32, name="ps_ao", tag="ao")
                for kc in range(CC):
                    nc.tensor.matmul(ps[:],
                                     wo_sb[kc][:, mc * 128:(mc + 1) * 128],
                                     ctxT_sb[kc][:],
                                     start=(kc == 0), stop=(kc == CC - 1))
                nc.vector.scalar_tensor_tensor(
                    r1_sb[mc][:], ps[:], prm(mc, P_BO), xtq_sb[mc][:],
                    mybir.AluOpType.add, mybir.AluOpType.add)
                nc.tensor.matmul(ps_sum[:], ones1_sb[:], r1_sb[mc][:],
                                 start=(mc == 0), stop=(mc == CC - 1))
                sq = bpool.tile([128, TQ], F32R, name="sq", tag="sq")
                nc.scalar.activation(sq[:], r1_sb[mc][:], ACTF.Square)
                nc.tensor.matmul(ps_sq[:], ones1_sb[:], sq[:],
                                 start=(mc == 0), stop=(mc == CC - 1))
            n = float(C)
            mean1 = bpool.tile([1, TQ], F32R, name="l1mean", tag="l1mean")
            nc.scalar.activation(mean1[:], ps_sum[:], ACTF.Copy, scale=1.0 / n)
            ex21 = bpool.tile([1, TQ], F32, name="l1ex2", tag="l1ex2")
            nc.scalar.activation(ex21[:], ps_sq[:], ACTF.Copy, scale=1.0 / n)
        hT_pool = tc.alloc_tile_pool(name="hTp", bufs=1, side="left")
        hT_sb = [hT_pool.tile([128, TQ], BF16, name=f"hT{cc}")
                 for cc in range(CC)]
        hT_holder["pool"] = hT_pool
        _ln_bcast(nc, bpool, eps_sb, lnr1, lnr2, mean1, ex21, "l1")
        with tc.tile_pool(name="pbc2", bufs=2, space="PSUM") as pbc2:
            for cc in range(CC):
                csl = slice(cc * 128, (cc + 1) * 128)
                pb2 = pbc2.tile([128, 2 * TQ], F32, name="lnpb", tag="bc")
                nc.tensor.matmul(pb2[:, 0:TQ], lnt1_sb[:, csl],
                                 lnr1[:], start=True, stop=True)
                nc.tensor.matmul(pb2[:, TQ:], lnt1_sb[:, csl],
                                 lnr2[:], start=True, stop=True)
                t1 = bpool.tile([128, TQ], F32, name="ln_t1", tag="lnt1")
                nc.vector.tensor_mul(t1[:], r1_sb[cc][:], pb2[:, 0:TQ])
                with nc.allow_low_precision(reason="bf16 activations"):
                    nc.vector.tensor_sub(hT_sb[cc][:], t1[:], pb2[:, TQ:])

    xtq_pool.release()

    # ==================== phase D: MLP + residual + LN2 =================
    with tc.tile_pool(name="dpool", bufs=2, side="right") as dpool, \
         tc.tile_pool(name="r2pool", bufs=1, side="right") as r2pool:

        r2_sb = [r2pool.tile([128, TQ], F32R, name=f"r2{cc}")
                 for cc in range(CC)]
        lnr1b = dpool.tile([65, TQ], F32R, name="lnr1b", tag="lnr1b")
        lnr2b = dpool.tile([65, TQ], F32R, name="lnr2b", tag="lnr2b")
        nc.gpsimd.dma_start(out=lnr1b[:], in_=d["lnz"][0, :, :])
        nc.gpsimd.dma_start(out=lnr2b[:], in_=d["lnz"][1, :, :])
        with tc.tile_pool(name="pfc2", bufs=1, space="PSUM") as pfc2:
            ps_m = [pfc2.tile([128, TQ], F32, name=f"ps_m{mc}", tag=f"m{mc}")
                    for mc in range(CC)]
            with tc.tile_pool(name="pfc1", bufs=2, space="PSUM") as pfc1:
                for kc2 in range(ICN):
                    jb = kc2 // CC
                    ps1 = pfc1.tile([128, TQ], F32, name="ps1", tag="f1")
                    co = (kc2 % CC) * 128
                    for kc in range(CC):
                        nc.tensor.matmul(
                            ps1[:], w1blk[jb][kc][:, co:co + 128],
                            hT_sb[kc][:],
                            start=(kc == 0), stop=(kc == CC - 1))
                    g = dpool.tile([128, TQ], BF16, name="g", tag="g")
                    with nc.allow_low_precision(reason="bf16 activations"):
                        nc.scalar.activation(g[:], ps1[:],
                                             ACTF.Gelu_apprx_tanh,
                                             bias=b1p_sb[:, kc2].unsqueeze(-1))
                    w2t = w2g[kc2 // CC][:, kc2 % CC, :]
                    for mc in range(CC):
                        nc.tensor.matmul(ps_m[mc][:],
                                         w2t[:, mc * 128:(mc + 1) * 128],
                                         g[:], start=(kc2 == 0),
                                         stop=(kc2 == ICN - 1))
                    # ring prefetches: issued after this iteration's readers
                    if kc2 % CC == CC - 1 and jb + 2 <= 3:
                        t = w1pool.tile([128, CC, C], BF16, name="w1_t",
                                        tag="w1")
                        nc.gpsimd.dma_start(out=t[:],
                                            in_=d["w1"][:, jb + 2, :, :])
                        w1blk[jb + 2] = [t[:, kc, :] for kc in range(CC)]
                        t2 = w2pool.tile([128, CC, C], BF16, name="w2_t",
                                         tag="w2")
                        nc.sync.dma_start(out=t2[:],
                                          in_=d["w2"][:, jb + 2, :, :])
                        w2g[jb + 2] = t2
            with tc.tile_pool(name="pst2", bufs=2, space="PSUM") as pst2:
                ps_sum2 = pst2.tile([1, TQ], F32, name="ps_sum2", tag="st")
                ps_sq2 = pst2.tile([1, TQ], F32, name="ps_sq2", tag="st")
                for mc in range(CC):
                    nc.vector.scalar_tensor_tensor(
                        r2_sb[mc][:], ps_m[mc][:], prm(mc, P_B2),
                        hT_sb[mc][:], mybir.AluOpType.add,
                        mybir.AluOpType.add)
                    nc.tensor.matmul(ps_sum2[:], ones1_sb[:], r2_sb[mc][:],
                                     start=(mc == 0), stop=(mc == CC - 1))
                    sq = dpool.tile([128, TQ], F32R, name="sq2", tag="sq")
                    nc.scalar.activation(sq[:], r2_sb[mc][:], ACTF.Square)
                    nc.tensor.matmul(ps_sq2[:], ones1_sb[:], sq[:],
                                     start=(mc == 0), stop=(mc == CC - 1))
                n = float(C)
                mean2 = dpool.tile([1, TQ], F32R, name="l2mean", tag="l2mean")
                nc.scalar.activation(mean2[:], ps_sum2[:], ACTF.Copy,
                                     scale=1.0 / n)
                ex22 = dpool.tile([1, TQ], F32, name="l2ex2", tag="l2ex2")
                nc.scalar.activation(ex22[:], ps_sq2[:], ACTF.Copy,
                                     scale=1.0 / n)
        hT_holder["pool"].release()
        ctxa_pool.release()
        _ln_bcast(nc, dpool, eps_sb, lnr1b, lnr2b, mean2, ex22, "l2")
        with tc.tile_pool(name="pbc3", bufs=2, space="PSUM") as pbc3:
            for cc in range(CC):
                csl = slice(cc * 128, (cc + 1) * 128)
                pb3 = pbc3.tile([128, 2 * TQ], F32, name="lnpb3", tag="bc")
                nc.tensor.matmul(pb3[:, 0:TQ], lnt2_sb[:, csl],
                                 lnr1b[:], start=True, stop=True)
                nc.tensor.matmul(pb3[:, TQ:], lnt2_sb[:, csl],
                                 lnr2b[:], start=True, stop=True)
                t1 = dpool.tile([128, TQ], F32, name="ln_t13", tag="lnt13")
                nc.vector.tensor_mul(t1[:], r2_sb[cc][:], pb3[:, 0:TQ])
                ot = dpool.tile([128, TQ], F32, name=f"o{cc}", tag=f"o{cc}",
                                bufs=1)
                nc.vector.tensor_sub(ot[:], t1[:], pb3[:, TQ:])
                nc.sync.dma_start(out=d["outT"][cc * 128:(cc + 1) * 128, :],
                                  in_=ot[:])

    w2pool.release()
    w1pool.release()
    wo_pool.release()
    ctxT_pool.release()
    const.release()


_NC = None


def _get_nc():
    global _NC
    if _NC is None:
        _NC = _build_nc()
    return _NC


def _prep_inmaps(x, Wq, bq, Wk, bk, Wv, bv, Wo, bo, ln1_s, ln1_b,
                 W1, b1, W2, b2, ln2_s, ln2_b):
    f32 = np.float32
    bf16 = ml_dtypes.bfloat16

    def pk(a):
        # [A*128, c...] -> [128, A, c...] contiguous (partition-major)
        a = np.asarray(a)
        return np.ascontiguousarray(
            a.reshape(-1, 128, *a.shape[1:]).swapaxes(0, 1))

    xT = [np.ascontiguousarray(np.asarray(x)[b].T, dtype=f32)
          for b in range(B)]
    xTh = [xb.astype(bf16) for xb in xT]
    wq = pk(np.asarray(Wq, f32).astype(bf16))
    # wk packed per-mc-slab: [128, mc, kc, 128]
    wk_p = pk(np.asarray(Wk, f32).astype(bf16))        # [128, kc, 768]
    wk = np.ascontiguousarray(
        wk_p.reshape(128, CC, CC, 128).swapaxes(1, 2))
    wv_p = pk(np.asarray(Wv, f32).astype(bf16))
    wva = np.ascontiguousarray(wv_p[:, :, 0:512])
    wvb = np.ascontiguousarray(wv_p[:, :, 512:768])
    wo = pk(np.asarray(Wo, f32).astype(bf16))
    w1f = np.asarray(W1, f32).astype(bf16)
    w1 = np.stack([pk(w1f[:, jb * C:(jb + 1) * C]) for jb in range(4)],
                  axis=1)
    w2f = np.asarray(W2, f32).astype(bf16)
    w2 = np.stack([pk(w2f[gg * C:(gg + 1) * C, :]) for gg in range(4)],
                  axis=1)
    # fold bv into bo: ctx_norm = ctx*rcp absorbs +bv exactly
    bo_f = np.asarray(bo, f32) + np.asarray(bv, f32) @ np.asarray(Wo, f32)
    prk = np.zeros((128, CC, 8), f32)
    for pi, arr in ((P_BQ, bq), (P_BK, bk), (P_BO, bo_f), (P_B2, b2),
                    (P_L1S, ln1_s), (P_L1B, ln1_b), (P_L2S, ln2_s),
                    (P_L2B, ln2_b)):
        prk[:, :, pi] = np.asarray(arr, f32).reshape(CC, 128).T
    b1p = np.ascontiguousarray(np.asarray(b1, f32).reshape(ICN, 128).T)
    lnt = np.zeros((2, 65, C), f32)
    lnt[0, 0], lnt[0, 32] = np.asarray(ln1_s, f32), -np.asarray(ln1_b, f32)
    lnt[1, 0], lnt[1, 32] = np.asarray(ln2_s, f32), -np.asarray(ln2_b, f32)
    selm = np.zeros((65, 3, DH), f32)
    for j in range(3):
        selm[32 * j, j, :] = 1.0
    ones = np.ones((128, H), f32)
    lnz = np.zeros((2, 65, TQ), f32)
    lnz[1, 32, :] = 1.0

    kk = np.arange(128)[:, None]
    in_maps = []
    for c in range(8):
        b, p = c // 4, c % 4
        qq = np.arange(32)[None, :]
        msk = np.ascontiguousarray(
            (kk <= 4 * qq + p).astype(bf16))  # k <= 4j+p, all chunks

        xtp = pk(xTh[b])  # [128, CC, T]
        xtp = np.ascontiguousarray(
            xtp.reshape(128, CC, TBN, 512).swapaxes(1, 2))
        in_maps.append({
            "xt": xtp,
            "xtqh": pk(np.ascontiguousarray(xTh[b][:, p::4])),
            "xtq": pk(np.ascontiguousarray(xT[b][:, p::4])),
            "wq": wq, "wk": wk, "wva": wva, "wvb": wvb, "wo": wo,
            "w1": w1, "w2": w2,
            "prk": prk, "b1p": b1p, "msk": msk,
            "lnt": lnt, "selm": selm, "ones": ones, "lnz": lnz,
        })
    return in_maps


def _run(in_maps, trace=False, **kw):
    nc = _get_nc()
    return run_bass_kernel_spmd(nc, in_maps, list(range(8)), trace=trace, **kw)


def kernel(**inputs):
    in_maps = _prep_inmaps(**inputs)
    res = _run(in_maps)
    out = np.empty((B, T, C), np.float32)
    for c in range(8):
        b, p = c // 4, c % 4
        out[b, p::4, :] = res.results[c]["outT"].T
    return out


# revision 35
# speedup vs baseline: 1.1204x; 1.0283x over previous
"""Trainium2 Bass kernel for one transformer block (B=2, T=2048, C=768, H=12,
inner=3072, fp32 I/O, causal, post-norm residual).

Sharding: 8 cores, token-interleaved. Core c handles batch c//4, tokens
p::4 (p = c%4) of that batch — every core runs the IDENTICAL program
(SPMD); causality is data-driven via per-core mask tensors.

v3 (vs 292us baseline): fused A' pipeline — scores+exp+ctx interleaved
INTO the K/V token-block loop so the ACT engine's ~60us of exp work (the
old phase-B bottleneck) hides under QKV matmuls:
- per tb: k, v computed; scores for the PREVIOUS tb's chunks run
  interleaved per head (score tiles -> exp -> boundary masks on the Pool
  engine -> ctx partial matmuls -> DVE accumulate into per-head f32
  SBUF accumulators ctxa [65,TQ]). PSUM: kv ring2 (2 banks) + v-ps2 (1)
  + score [128,1024] ring2 (4) + ctx [65,512] ring1 (1) = 8 exactly.
- q is kc-outer (6 psum banks, before the tb loop) with per-chunk
  wq/xtqh DMAs so the first matmul fires after ~330KB lands (was ~2MB).
- bv folded host-side into bo' = bo + bv @ Wo (ctx_norm = ctx*rcp
  absorbs +bv exactly via the denominator row): v eviction is a pure
  copy, no bias tile.
- softmax reciprocal = exp(-ln(dn)) on ACT (idle in B'), per head; the
  old 14.6us DVE reciprocal is gone. K=65 selector matmul broadcasts
  rcp; the psum->sbuf ctx eviction fuses into the normalize multiply.
- wk packed per-mc-slab so k(mc) streams as slabs land; wv split
  512/256; w1/w2 + xtq stream during B' when qz/kT/v free up.
"""

import sys

if "/opt/trn_rl_repo" not in sys.path:
    sys.path.insert(0, "/opt/trn_rl_repo")

import numpy as np
import ml_dtypes

import concourse.bacc as bacc
import concourse.mybir as mybir
import concourse.tile as tile
from concourse.bass_utils import run_bass_kernel_spmd

F32 = mybir.dt.float32
F32R = mybir.dt.float32r
BF16 = mybir.dt.bfloat16
ACTF = mybir.ActivationFunctionType

B, T, C = 2, 2048, 768
H, DH = 12, 64
IN = 3072
CC = C // 128          # 6 channel chunks
TBN = T // 512         # 4 token blocks of full seq
TQ = 512               # tokens per core
KCN = T // 128         # 16 k-chunks
ICN = IN // 128        # 24 inner chunks
EPS = 1e-4
SCALE = 1.0 / np.sqrt(DH)

# param pack order in "prk" [128, CC, 8]
P_BQ, P_BK, P_BO, P_B2, P_L1S, P_L1B, P_L2S, P_L2B = range(8)

# score-tile packing per tb: tiles of (global kc chunk, col offset).
# Chunk kc covers queries 32*kc..TQ (width TQ-32*kc). Offsets chosen so
# NO matmul output crosses a 2KB psum bank boundary (512 f32 cols) —
# a crossing write half-overwrites / half-accumulates-onto-stale.
# Gaps between chunks get exp'd (garbage, never read) — harmless.
SPACK = [
    [[(0, 0), (1, 512)], [(2, 0), (3, 512)]],
    [[(4, 0), (5, 512)], [(6, 0), (7, 512)]],
    [[(8, 0), (9, 256), (10, 512), (11, 704)]],
    [[(12, 0), (13, 128), (14, 224), (15, 288)]],
]


def _build_nc():
    nc = bacc.Bacc("TRN2", target_bir_lowering=False, debug=False,
                   enable_asserts=False, num_devices=8)
    d = {}
    d["xt"] = nc.dram_tensor("xt", [128, TBN, CC, 512], BF16,
                             kind="ExternalInput").ap()
    d["xtqh"] = nc.dram_tensor("xtqh", [128, CC, TQ], BF16,
                               kind="ExternalInput").ap()
    d["xtq"] = nc.dram_tensor("xtq", [128, CC, TQ], F32R,
                              kind="ExternalInput").ap()
    d["wq"] = nc.dram_tensor("wq", [128, CC, C], BF16,
                             kind="ExternalInput").ap()
    d["wk"] = nc.dram_tensor("wk", [128, CC, CC, 128], BF16,
                             kind="ExternalInput").ap()  # [p, mc, kc, j]
    d["wva"] = nc.dram_tensor("wva", [128, CC, 512], BF16,
                              kind="ExternalInput").ap()
    d["wvb"] = nc.dram_tensor("wvb", [128, CC, 256], BF16,
                              kind="ExternalInput").ap()
    d["wo"] = nc.dram_tensor("wo", [128, CC, C], BF16,
                             kind="ExternalInput").ap()
    d["w1"] = nc.dram_tensor("w1", [128, 4, CC, C], BF16,
                             kind="ExternalInput").ap()
    d["w2"] = nc.dram_tensor("w2", [128, 4, CC, C], BF16,
                             kind="ExternalInput").ap()
    d["prk"] = nc.dram_tensor("prk", [128, CC, 8], F32, kind="ExternalInput").ap()
    d["b1p"] = nc.dram_tensor("b1p", [128, ICN], F32, kind="ExternalInput").ap()
    d["msk"] = nc.dram_tensor("msk", [128, 32], BF16, kind="ExternalInput").ap()
    d["ones"] = nc.dram_tensor("ones", [128, H], F32R, kind="ExternalInput").ap()
    d["lnz"] = nc.dram_tensor("lnz", [2, 65, TQ], F32R, kind="ExternalInput").ap()
    d["selm"] = nc.dram_tensor("selm", [65, 3, DH], F32R, kind="ExternalInput").ap()
    d["lnt"] = nc.dram_tensor("lnt", [2, 65, C], F32R, kind="ExternalInput").ap()
    d["outT"] = nc.dram_tensor("outT", [C, TQ], F32, kind="ExternalOutput").ap()

    with tile.TileContext(nc) as tc:
        _emit(nc, tc, d)
    nc.finalize()
    return nc


def _ln_bcast(nc, pool, eps_sb, lnr1, lnr2, mean, ex2, tagp):
    """Fill lnr1 (row0 = istd) and lnr2 (row0 = mean*istd, row32 = ones)
    so the LN broadcasts run as K=65 selector matmuls at full PE rate.
    istd = exp(-0.5*ln(var+eps)) on ACT."""
    n = float(C)
    m2 = pool.tile([1, TQ], F32, name="ln_m2", tag=tagp + "m2")
    nc.vector.tensor_mul(m2[:], mean[:], mean[:])
    dv = pool.tile([1, TQ], F32, name="ln_d", tag=tagp + "d")
    nc.vector.tensor_sub(dv[:], ex2[:], m2[:])
    lnv = pool.tile([1, TQ], F32, name="ln_lnv", tag=tagp + "lnv")
    nc.scalar.activation(lnv[:], dv[:], ACTF.Ln,
                         scale=n / (n - 1.0), bias=eps_sb[:])
    with nc.allow_low_precision(reason="f32r matmul operand"):
        nc.scalar.activation(lnr1[0:1, :], lnv[:], ACTF.Exp, scale=-0.5)
        nc.vector.tensor_mul(lnr2[0:1, :], mean[:], lnr1[0:1, :])


def _emit(nc, tc, d):
    # ---- persistent constants ------------------------------------------
    const = tc.alloc_tile_pool(name="const", bufs=1, side="left")
    onesh_sb = const.tile([128, H], F32R, name="onesh_sb")
    selm_sb = const.tile([65, 3, DH], F32R, name="selm_sb")
    lnt1_sb = const.tile([65, C], F32R, name="lnt1_sb")
    lnt2_sb = const.tile([65, C], F32R, name="lnt2_sb")
    eps_sb = const.tile([1, 1], F32, name="eps_sb")
    prk_sb = const.tile([128, CC, 8], F32, name="prk_sb")
    b1p_sb = const.tile([128, ICN], F32, name="b1p_sb")
    mskb_sb = const.tile([128, 32], BF16, name="mskb_sb")

    def prm(cc, pi):
        return prk_sb[:, cc, pi].unsqueeze(-1)  # [128,1]

    nc.vector.memset(eps_sb[:], float(EPS))
    nc.gpsimd.dma_start(out=onesh_sb[:], in_=d["ones"][:])
    ones1_sb = onesh_sb[:, 0:1]
    nc.gpsimd.dma_start(out=prk_sb[:], in_=d["prk"][:])
    nc.gpsimd.dma_start(out=mskb_sb[:], in_=d["msk"][:])
    nc.gpsimd.dma_start(out=selm_sb[:], in_=d["selm"][:])
    nc.gpsimd.dma_start(out=lnt1_sb[:], in_=d["lnt"][0, :, :])
    nc.gpsimd.dma_start(out=lnt2_sb[:], in_=d["lnt"][1, :, :])
    nc.gpsimd.dma_start(out=b1p_sb[:], in_=d["b1p"][:])

    # ---- persistent activation tensors ---------------------------------
    # left-side stack order = reverse release order: ctxT/ctxa released
    # late, qT/kv right after A'
    ctxT_pool = tc.alloc_tile_pool(name="ctxTp", bufs=1, side="left")
    ctxT_sb = [ctxT_pool.tile([128, TQ], BF16, name=f"ctxT{cc}")
               for cc in range(CC)]
    ctxa_pool = tc.alloc_tile_pool(name="ctxap", bufs=1, side="left")
    ctxa_sb = [ctxa_pool.tile([DH + 1, TQ], F32, name=f"ctxa{h}")
               for h in range(H)]
    qT_pool = tc.alloc_tile_pool(name="qTp", bufs=1, side="left")
    qz_sb = [qT_pool.tile([128, TQ], BF16, name=f"qz{h}") for h in range(H)]
    for h in range(H):
        ro = (h % 2) * DH
        with nc.allow_low_precision(reason="zero fill"):
            nc.vector.memset(qz_sb[h][(DH - ro):(128 - ro), :], 0.0)

    kv_pool = tc.alloc_tile_pool(name="kvp", bufs=1, side="left")
    kT_sb = [kv_pool.tile([128, T], BF16, name=f"kT{cc}") for cc in range(CC)]
    v_sb = [kv_pool.tile([128, H, DH + 1], BF16, name=f"v{tch}")
            for tch in range(KCN)]
    for tch in range(KCN):
        with nc.allow_low_precision(reason="ones fill"):
            nc.vector.tensor_copy(v_sb[tch][:, :, DH], onesh_sb[:])

    # ==================== phase A': QKV + scores + exp + ctx ============
    wo_pool = tc.alloc_tile_pool(name="wop", bufs=1, side="right")
    with tc.tile_pool(name="xqs", bufs=1, side="right") as xq_pool, \
         tc.tile_pool(name="wqs", bufs=1, side="right") as wq_pool, \
         tc.tile_pool(name="wkvs", bufs=1, side="right") as wkv_pool, \
         tc.tile_pool(name="xts", bufs=2, side="right") as xt_pool, \
         tc.tile_pool(name="etp", bufs=1, side="right") as et_pool:

        # -- DMA schedule ------------------------------------------------
        # sync q: xtqh chunks, xt blocks, wva/wvb
        # scalar q: wq chunks, wk mc-slabs
        # gpsimd q: small consts (above), wo
        xtqh_pk = xq_pool.tile([128, CC, TQ], BF16, name="xtqh_pk")
        wq_pk = wq_pool.tile([128, CC, C], BF16, name="wq_pk")
        for kc in range(CC):
            nc.sync.dma_start(out=xtqh_pk[:, kc, :], in_=d["xtqh"][:, kc, :])
            nc.scalar.dma_start(out=wq_pk[:, kc, :], in_=d["wq"][:, kc, :])
        wk_pk = wkv_pool.tile([128, CC, CC, 128], BF16, name="wk_pk")
        for mc in range(CC):
            nc.scalar.dma_start(out=wk_pk[:, mc, :, :],
                                in_=d["wk"][:, mc, :, :])
        xt_tiles = {}
        for tb in range(2):
            t = xt_pool.tile([128, CC, 512], BF16, name="xt_t", tag="xt")
            nc.sync.dma_start(out=t[:], in_=d["xt"][:, tb, :, :])
            xt_tiles[tb] = t
        wva_pk = wkv_pool.tile([128, CC, 512], BF16, name="wva_pk")
        nc.scalar.dma_start(out=wva_pk[:], in_=d["wva"][:])
        wvb_pk = wkv_pool.tile([128, CC, 256], BF16, name="wvb_pk")
        nc.scalar.dma_start(out=wvb_pk[:], in_=d["wvb"][:])
        wo_pk = wo_pool.tile([128, CC, C], BF16, name="wo_pk")
        nc.gpsimd.dma_start(out=wo_pk[:], in_=d["wo"][:])
        wo_sb = [wo_pk[:, cc, :] for cc in range(CC)]

        # -- q: kc-outer so first matmul needs only chunk 0 of wq/xtqh --
        with tc.tile_pool(name="pq", bufs=1, space="PSUM") as pq:
            ps_q = [pq.tile([128, TQ], F32, name=f"psq{mc}", tag=f"q{mc}")
                    for mc in range(CC)]
            for kc in range(CC):
                for mc in range(CC):
                    nc.tensor.matmul(ps_q[mc][:],
                                     wq_pk[:, kc, mc * 128:(mc + 1) * 128],
                                     xtqh_pk[:, kc, :],
                                     start=(kc == 0), stop=(kc == CC - 1))
            for mc in range(CC):
                with nc.allow_low_precision(reason="bf16 activations"):
                    nc.vector.tensor_scalar_add(
                        qz_sb[2 * mc][0:DH, :], ps_q[mc][0:DH, :],
                        prm(mc, P_BQ)[0:DH, :])
                    nc.vector.tensor_scalar_add(
                        qz_sb[2 * mc + 1][DH:128, :], ps_q[mc][DH:128, :],
                        prm(mc, P_BQ)[DH:128, :])

        with tc.tile_pool(name="psc", bufs=1, space="PSUM") as psc, \
             tc.tile_pool(name="pctx", bufs=1, space="PSUM") as pctx:

            cur_et = {}   # h -> list of (et_tile, off, w, kc) for this tb

            def emit_scores(tb, h):
                cc = h // 2
                kTh = kT_sb[cc]
                qzh = qz_sb[h]
                lst = []
                for kcs in SPACK[tb]:
                    tw = kcs[-1][1] + TQ - 32 * kcs[-1][0]
                    ps = psc.tile([128, 1024], F32, name="ps_s",
                                  tag="sA", bufs=2)
                    for kc, off in kcs:
                        nc.tensor.matmul(
                            ps[:, off:off + TQ - 32 * kc],
                            kTh[:, kc * 128:(kc + 1) * 128],
                            qzh[:, 32 * kc:TQ],
                            start=True, stop=True)
                    et = et_pool.tile([128, 1024], BF16, name="et",
                                      tag="et", bufs=6)
                    nc.scalar.activation(et[:, 0:tw], ps[:, 0:tw], ACTF.Exp,
                                         scale=float(SCALE))
                    for kc, o in kcs:
                        with nc.allow_low_precision(reason="bf16 mask"):
                            nc.vector.tensor_mul(
                                et[:, o:o + 32], et[:, o:o + 32],
                                mskb_sb[:])
                        lst.append((et, o, TQ - 32 * kc, kc))
                cur_et[h] = lst

            def emit_ctx(tb, h):
                ps = pctx.tile([DH + 1, TQ], F32, name="ctx_ps", tag="ctx",
                               bufs=1)
                lst = cur_et.pop(h)
                w0 = lst[0][2]
                for i, (et, o, w, kc) in enumerate(lst):
                    nc.tensor.matmul(ps[:, TQ - w:], v_sb[kc][:, h, :],
                                     et[:, o:o + w],
                                     start=(i == 0), stop=(i == len(lst) - 1))
                if tb == 0:
                    nc.vector.tensor_copy(ctxa_sb[h][:], ps[:])
                else:
                    nc.vector.tensor_add(ctxa_sb[h][:, TQ - w0:],
                                         ctxa_sb[h][:, TQ - w0:],
                                         ps[:, TQ - w0:])

            # windows 0-2: scores/exp/ctx of tb interleaved with k/v(tb+1)
            with tc.tile_pool(name="pkv", bufs=1, space="PSUM") as pkv:

                def emit_k(tb, mc):
                    xt_blk = xt_tiles[tb]
                    ps = pkv.tile([128, 512], F32, name="ps_k", tag="kv1",
                                  bufs=2)
                    for kc in range(CC):
                        nc.tensor.matmul(
                            ps[:], wk_pk[:, mc, kc, :], xt_blk[:, kc, :],
                            start=(kc == 0), stop=(kc == CC - 1))
                    with nc.allow_low_precision(reason="bf16 activations"):
                        nc.vector.tensor_scalar_add(
                            kT_sb[mc][:, tb * 512:(tb + 1) * 512], ps[:],
                            prm(mc, P_BK))

                def emit_v(tb, tci):
                    xt_blk = xt_tiles[tb]
                    tch = tb * 4 + tci
                    ps1 = pkv.tile([128, 512], F32, name="ps_v1", tag="kv1",
                                   bufs=2)
                    ps2 = pkv.tile([128, 256], F32, name="ps_v2", tag="v2",
                                   bufs=1)
                    for kc in range(CC):
                        xsl = xt_blk[:, kc, tci * 128:(tci + 1) * 128]
                        nc.tensor.matmul(ps1[:], xsl, wva_pk[:, kc, :],
                                         start=(kc == 0), stop=(kc == CC - 1))
                        nc.tensor.matmul(ps2[:], xsl, wvb_pk[:, kc, :],
                                         start=(kc == 0), stop=(kc == CC - 1))
                    vt = v_sb[tch]
                    with nc.allow_low_precision(reason="bf16 activations"):
                        nc.vector.tensor_copy(
                            vt[:, 0:8, 0:DH],
                            ps1[:].rearrange("p (h d) -> p h d", d=DH))
                        nc.vector.tensor_copy(
                            vt[:, 8:H, 0:DH],
                            ps2[:].rearrange("p (h d) -> p h d", d=DH))

                # tb0 k/v (nothing to interleave yet)
                for mc in range(CC):
                    emit_k(0, mc)
                for tci in range(4):
                    emit_v(0, tci)

                for stb in range(TBN - 1):
                    ftb = stb + 1
                    if ftb + 1 < TBN:
                        t2 = xt_pool.tile([128, CC, 512], BF16,
                                          name="xt_t", tag="xt")
                        nc.sync.dma_start(out=t2[:],
                                          in_=d["xt"][:, ftb + 1, :, :])
                        xt_tiles[ftb + 1] = t2
                        del t2
                    fills = [(emit_k, (ftb, mc)) for mc in range(CC)]
                    fills += [(emit_v, (ftb, tci)) for tci in range(4)]
                    for h in range(H):
                        emit_scores(stb, h)
                        if h > 0:
                            emit_ctx(stb, h - 1)
                        if h < len(fills):
                            fn, args = fills[h]
                            fn(*args)
                    emit_ctx(stb, H - 1)

            # window 3: tb3 scores/ctx with softmax normalization fused in
            # (kv banks freed above; pb broadcasts use them). rcp on DVE —
            # ln/exp tables are in different ACT sets, a per-head ln/exp
            # reciprocal costs 2 table loads (1.28us each).
            with tc.tile_pool(name="ppb", bufs=1, space="PSUM") as ppb:
                dnb = [et_pool.tile([65, TQ], F32, name=f"dnb{i}")
                       for i in range(2)]
                nc.vector.memset(dnb[0][:], 1.0)
                nc.vector.memset(dnb[1][:], 1.0)

                def emit_norm(bi):
                    dnt = dnb[bi % 2]
                    for j in range(3):
                        h = 3 * bi + j
                        nc.vector.tensor_copy(dnt[32 * j:32 * j + 1, :],
                                              ctxa_sb[h][DH:DH + 1, :])
                    rcpf = et_pool.tile([65, TQ], F32, name="rcpf",
                                        tag="rcpf", bufs=2)
                    nc.vector.reciprocal_approx_fast(rcpf[:], dnt[:])
                    rcp = et_pool.tile([65, TQ], F32R, name="rcp",
                                       tag="rcp", bufs=2)
                    with nc.allow_low_precision(reason="f32r operand"):
                        nc.vector.tensor_copy(rcp[:], rcpf[:])
                    for j in range(3):
                        h = 3 * bi + j
                        cc, ro = h // 2, (h % 2) * DH
                        pb = ppb.tile([DH, TQ], F32, name="pb", tag="pb",
                                      bufs=2)
                        nc.tensor.matmul(pb[:], selm_sb[:, j, :], rcp[:],
                                         start=True, stop=True)
                        with nc.allow_low_precision(reason="bf16 out"):
                            nc.vector.tensor_mul(
                                ctxT_sb[cc][ro:ro + DH, :],
                                ctxa_sb[h][0:DH, :], pb[:])

                for h in range(H):
                    emit_scores(3, h)
                    if h > 0:
                        emit_ctx(3, h - 1)
                    if h in (4, 7, 10):
                        emit_norm((h - 4) // 3)
                emit_ctx(3, H - 1)
                emit_norm(3)

    kv_pool.release()
    qT_pool.release()

    # ==================== phase B': norm + Wo + LN1 =====================
    w1pool = tc.alloc_tile_pool(name="w1pool", bufs=2, side="right")
    w2pool = tc.alloc_tile_pool(name="w2pool", bufs=2, side="right")
    xtq_pool = tc.alloc_tile_pool(name="xtqp", bufs=1, side="right")

    hT_holder = {}
    with tc.tile_pool(name="bpool", bufs=2, side="right") as bpool, \
         tc.tile_pool(name="r1pool", bufs=1, side="right") as r1pool:

        # streams that waited on qz/kT/v SBUF space
        xtq_pk = xtq_pool.tile([128, CC, TQ], F32R, name="xtq_pk")
        nc.sync.dma_start(out=xtq_pk[:], in_=d["xtq"][:])
        xtq_sb = [xtq_pk[:, cc, :] for cc in range(CC)]
        w1blk = {}
        for jb in range(2):
            t = w1pool.tile([128, CC, C], BF16, name="w1_t", tag="w1")
            nc.sync.dma_start(out=t[:], in_=d["w1"][:, jb, :, :])
            w1blk[jb] = [t[:, kc, :] for kc in range(CC)]
        w2g = {}
        for gg in range(2):
            t = w2pool.tile([128, CC, C], BF16, name="w2_t", tag="w2")
            nc.sync.dma_start(out=t[:], in_=d["w2"][:, gg, :, :])
            w2g[gg] = t

        # preload the LN act table off the critical LN1 chain
        dmy = bpool.tile([1, 1], F32, name="dmy", tag="dmy")
        nc.scalar.activation(dmy[:], eps_sb[:], ACTF.Ln)

        # Wo + residual + LN1 stats
        r1_sb = [r1pool.tile([128, TQ], F32R, name=f"r1{cc}")
                 for cc in range(CC)]
        lnr1 = bpool.tile([65, TQ], F32R, name="lnr1", tag="lnr1")
        lnr2 = bpool.tile([65, TQ], F32R, name="lnr2", tag="lnr2")
        nc.sync.dma_start(out=lnr1[:], in_=d["lnz"][0, :, :])
        nc.sync.dma_start(out=lnr2[:], in_=d["lnz"][1, :, :])
        with tc.tile_pool(name="pao", bufs=2, space="PSUM") as pao, \
             tc.tile_pool(name="pst", bufs=2, space="PSUM") as pst:
            ps_sum = pst.tile([1, TQ], F32, name="ps_sum", tag="st")
            ps_sq = pst.tile([1, TQ], F32, name="ps_sq", tag="st")
            for mc in range(CC):
                ps = pao.tile([128, TQ], F32, name="ps_ao", tag="ao")
                for kc in range(CC):
                    nc.tensor.matmul(ps[:],
                                     wo_sb[kc][:, mc * 128:(mc + 1) * 128],
                                     ctxT_sb[kc][:],
                                     start=(kc == 0), stop=(kc == CC - 1))
                nc.vector.scalar_tensor_tensor(
                    r1_sb[mc][:], ps[:], prm(mc, P_BO), xtq_sb[mc][:],
                    mybir.AluOpType.add, mybir.AluOpType.add)
                nc.tensor.matmul(ps_sum[:], ones1_sb[:], r1_sb[mc][:],
                                 start=(mc == 0), stop=(mc == CC - 1))
                sq = bpool.tile([128, TQ], F32R, name="sq", tag="sq")
                nc.scalar.activation(sq[:], r1_sb[mc][:], ACTF.Square)
                nc.tensor.matmul(ps_sq[:], ones1_sb[:], sq[:],
                                 start=(mc == 0), stop=(mc == CC - 1))
            n = float(C)
            mean1 = bpool.tile([1, TQ], F32R, name="l1mean", tag="l1mean")
            nc.scalar.activation(mean1[:], ps_sum[:], ACTF.Copy, scale=1.0 / n)
            ex21 = bpool.tile([1, TQ], F32, name="l1ex2", tag="l1ex2")
            nc.scalar.activation(ex21[:], ps_sq[:], ACTF.Copy, scale=1.0 / n)
        hT_pool = tc.alloc_tile_pool(name="hTp", bufs=1, side="left")
        hT_sb = [hT_pool.tile([128, TQ], BF16, name=f"hT{cc}")
                 for cc in range(CC)]
        hT_holder["pool"] = hT_pool
        _ln_bcast(nc, bpool, eps_sb, lnr1, lnr2, mean1, ex21, "l1")
        nc.scalar.activation(dmy[:], eps_sb[:], ACTF.Gelu_apprx_tanh)
        with tc.tile_pool(name="pbc2", bufs=2, space="PSUM") as pbc2:
            for cc in range(CC):
                csl = slice(cc * 128, (cc + 1) * 128)
                pb2 = pbc2.tile([128, 2 * TQ], F32, name="lnpb", tag="bc")
                nc.tensor.matmul(pb2[:, 0:TQ], lnt1_sb[:, csl],
                                 lnr1[:], start=True, stop=True)
                nc.tensor.matmul(pb2[:, TQ:], lnt1_sb[:, csl],
                                 lnr2[:], start=True, stop=True)
                t1 = bpool.tile([128, TQ], F32, name="ln_t1", tag="lnt1")
                nc.vector.tensor_mul(t1[:], r1_sb[cc][:], pb2[:, 0:TQ])
                with nc.allow_low_precision(reason="bf16 activations"):
                    nc.vector.tensor_sub(hT_sb[cc][:], t1[:], pb2[:, TQ:])

    xtq_pool.release()

    # ==================== phase D: MLP + residual + LN2 =================
    with tc.tile_pool(name="dpool", bufs=2, side="right") as dpool, \
         tc.tile_pool(name="r2pool", bufs=1, side="right") as r2pool:

        r2_sb = [r2pool.tile([128, TQ], F32R, name=f"r2{cc}")
                 for cc in range(CC)]
        lnr1b = dpool.tile([65, TQ], F32R, name="lnr1b", tag="lnr1b")
        lnr2b = dpool.tile([65, TQ], F32R, name="lnr2b", tag="lnr2b")
        nc.sync.dma_start(out=lnr1b[:], in_=d["lnz"][0, :, :])
        nc.sync.dma_start(out=lnr2b[:], in_=d["lnz"][1, :, :])
        with tc.tile_pool(name="pfc2", bufs=1, space="PSUM") as pfc2:
            ps_m = [pfc2.tile([128, TQ], F32, name=f"ps_m{mc}", tag=f"m{mc}")
                    for mc in range(CC)]
            with tc.tile_pool(name="pfc1", bufs=2, space="PSUM") as pfc1:
                for kc2 in range(ICN):
                    jb = kc2 // CC
                    ps1 = pfc1.tile([128, TQ], F32, name="ps1", tag="f1")
                    co = (kc2 % CC) * 128
                    for kc in range(CC):
                        nc.tensor.matmul(
                            ps1[:], w1blk[jb][kc][:, co:co + 128],
                            hT_sb[kc][:],
                            start=(kc == 0), stop=(kc == CC - 1))
                    g = dpool.tile([128, TQ], BF16, name="g", tag="g")
                    with nc.allow_low_precision(reason="bf16 activations"):
                        nc.scalar.activation(g[:], ps1[:],
                                             ACTF.Gelu_apprx_tanh,
                                             bias=b1p_sb[:, kc2].unsqueeze(-1))
                    w2t = w2g[kc2 // CC][:, kc2 % CC, :]
                    for mc in range(CC):
                        nc.tensor.matmul(ps_m[mc][:],
                                         w2t[:, mc * 128:(mc + 1) * 128],
                                         g[:], start=(kc2 == 0),
                                         stop=(kc2 == ICN - 1))
                    # ring prefetches: issued after this iteration's readers
                    if kc2 % CC == CC - 1 and jb + 2 <= 3:
                        t = w1pool.tile([128, CC, C], BF16, name="w1_t",
                                        tag="w1")
                        nc.sync.dma_start(out=t[:],
                                          in_=d["w1"][:, jb + 2, :, :])
                        w1blk[jb + 2] = [t[:, kc, :] for kc in range(CC)]
                        t2 = w2pool.tile([128, CC, C], BF16, name="w2_t",
                                         tag="w2")
                        nc.sync.dma_start(out=t2[:],
                                          in_=d["w2"][:, jb + 2, :, :])
                        w2g[jb + 2] = t2
            dmy2 = dpool.tile([1, 1], F32, name="dmy2", tag="dmy2")
            nc.scalar.activation(dmy2[:], eps_sb[:], ACTF.Ln)
            with tc.tile_pool(name="pst2", bufs=2, space="PSUM") as pst2:
                ps_sum2 = pst2.tile([1, TQ], F32, name="ps_sum2", tag="st")
                ps_sq2 = pst2.tile([1, TQ], F32, name="ps_sq2", tag="st")
                for mc in range(CC):
                    nc.vector.scalar_tensor_tensor(
                        r2_sb[mc][:], ps_m[mc][:], prm(mc, P_B2),
                        hT_sb[mc][:], mybir.AluOpType.add,
                        mybir.AluOpType.add)
                    nc.tensor.matmul(ps_sum2[:], ones1_sb[:], r2_sb[mc][:],
                                     start=(mc == 0), stop=(mc == CC - 1))
                    sq = dpool.tile([128, TQ], F32R, name="sq2", tag="sq")
                    nc.scalar.activation(sq[:], r2_sb[mc][:], ACTF.Square)
                    nc.tensor.matmul(ps_sq2[:], ones1_sb[:], sq[:],
                                     start=(mc == 0), stop=(mc == CC - 1))
                n = float(C)
                mean2 = dpool.tile([1, TQ], F32R, name="l2mean", tag="l2mean")
                nc.scalar.activation(mean2[:], ps_sum2[:], ACTF.Copy,
                                     scale=1.0 / n)
                ex22 = dpool.tile([1, TQ], F32, name="l2ex2", tag="l2ex2")
                nc.scalar.activation(ex22[:], ps_sq2[:], ACTF.Copy,
                                     scale=1.0 / n)
        hT_holder["pool"].release()
        ctxa_pool.release()
        _ln_bcast(nc, dpool, eps_sb, lnr1b, lnr2b, mean2, ex22, "l2")
        with tc.tile_pool(name="pbc3", bufs=2, space="PSUM") as pbc3:
            for cc in range(CC):
                csl = slice(cc * 128, (cc + 1) * 128)
                pb3 = pbc3.tile([128, 2 * TQ], F32, name="lnpb3", tag="bc")
                nc.tensor.matmul(pb3[:, 0:TQ], lnt2_sb[:, csl],
                                 lnr1b[:], start=True, stop=True)
                nc.tensor.matmul(pb3[:, TQ:], lnt2_sb[:, csl],
                                 lnr2b[:], start=True, stop=True)
                t1 = dpool.tile([128, TQ], F32, name="ln_t13", tag="lnt13")
                nc.vector.tensor_mul(t1[:], r2_sb[cc][:], pb3[:, 0:TQ])
                ot = dpool.tile([128, TQ], F32, name=f"o{cc}", tag=f"o{cc}",
                                bufs=1)
                nc.vector.tensor_sub(ot[:], t1[:], pb3[:, TQ:])
                nc.sync.dma_start(out=d["outT"][cc * 128:(cc + 1) * 128, :],
                                  in_=ot[:])

    w2pool.release()
    w1pool.release()
    wo_pool.release()
    ctxT_pool.release()
    const.release()


_NC = None


def _get_nc():
    global _NC
    if _NC is None:
        _NC = _build_nc()
    return _NC


def _prep_inmaps(x, Wq, bq, Wk, bk, Wv, bv, Wo, bo, ln1_s, ln1_b,
                 W1, b1, W2, b2, ln2_s, ln2_b):
    f32 = np.float32
    bf16 = ml_dtypes.bfloat16

    def pk(a):
        # [A*128, c...] -> [128, A, c...] contiguous (partition-major)
        a = np.asarray(a)
        return np.ascontiguousarray(
            a.reshape(-1, 128, *a.shape[1:]).swapaxes(0, 1))

    xT = [np.ascontiguousarray(np.asarray(x)[b].T, dtype=f32)
          for b in range(B)]
    xTh = [xb.astype(bf16) for xb in xT]
    wq = pk(np.asarray(Wq, f32).astype(bf16))
    # wk packed per-mc-slab: [128, mc, kc, 128]
    wk_p = pk(np.asarray(Wk, f32).astype(bf16))        # [128, kc, 768]
    wk = np.ascontiguousarray(
        wk_p.reshape(128, CC, CC, 128).swapaxes(1, 2))
    wv_p = pk(np.asarray(Wv, f32).astype(bf16))
    wva = np.ascontiguousarray(wv_p[:, :, 0:512])
    wvb = np.ascontiguousarray(wv_p[:, :, 512:768])
    wo = pk(np.asarray(Wo, f32).astype(bf16))
    w1f = np.asarray(W1, f32).astype(bf16)
    w1 = np.stack([pk(w1f[:, jb * C:(jb + 1) * C]) for jb in range(4)],
                  axis=1)
    w2f = np.asarray(W2, f32).astype(bf16)
    w2 = np.stack([pk(w2f[gg * C:(gg + 1) * C, :]) for gg in range(4)],
                  axis=1)
    # fold bv into bo: ctx_norm = ctx*rcp absorbs +bv exactly
    bo_f = np.asarray(bo, f32) + np.asarray(bv, f32) @ np.asarray(Wo, f32)
    prk = np.zeros((128, CC, 8), f32)
    for pi, arr in ((P_BQ, bq), (P_BK, bk), (P_BO, bo_f), (P_B2, b2),
                    (P_L1S, ln1_s), (P_L1B, ln1_b), (P_L2S, ln2_s),
                    (P_L2B, ln2_b)):
        prk[:, :, pi] = np.asarray(arr, f32).reshape(CC, 128).T
    b1p = np.ascontiguousarray(np.asarray(b1, f32).reshape(ICN, 128).T)
    lnt = np.zeros((2, 65, C), f32)
    lnt[0, 0], lnt[0, 32] = np.asarray(ln1_s, f32), -np.asarray(ln1_b, f32)
    lnt[1, 0], lnt[1, 32] = np.asarray(ln2_s, f32), -np.asarray(ln2_b, f32)
    selm = np.zeros((65, 3, DH), f32)
    for j in range(3):
        selm[32 * j, j, :] = 1.0
    ones = np.ones((128, H), f32)
    lnz = np.zeros((2, 65, TQ), f32)
    lnz[1, 32, :] = 1.0

    kk = np.arange(128)[:, None]
    in_maps = []
    for c in range(8):
        b, p = c // 4, c % 4
        qq = np.arange(32)[None, :]
        msk = np.ascontiguousarray(
            (kk <= 4 * qq + p).astype(bf16))  # k <= 4j+p, all chunks

        xtp = pk(xTh[b])  # [128, CC, T]
        xtp = np.ascontiguousarray(
            xtp.reshape(128, CC, TBN, 512).swapaxes(1, 2))
        in_maps.append({
            "xt": xtp,
            "xtqh": pk(np.ascontiguousarray(xTh[b][:, p::4])),
            "xtq": pk(np.ascontiguousarray(xT[b][:, p::4])),
            "wq": wq, "wk": wk, "wva": wva, "wvb": wvb, "wo": wo,
            "w1": w1, "w2": w2,
            "prk": prk, "b1p": b1p, "msk": msk,
            "lnt": lnt, "selm": selm, "ones": ones, "lnz": lnz,
        })
    return in_maps


def _run(in_maps, trace=False, **kw):
    nc = _get_nc()
    return run_bass_kernel_spmd(nc, in_maps, list(range(8)), trace=trace, **kw)


def kernel(**inputs):
    in_maps = _prep_inmaps(**inputs)
    res = _run(in_maps)
    out = np.empty((B, T, C), np.float32)
    for c in range(8):
        b, p = c // 4, c % 4
        out[b, p::4, :] = res.results[c]["outT"].T
    return out


# revision 36
# speedup vs baseline: 1.1296x; 1.0082x over previous
"""Trainium2 Bass kernel for one transformer block (B=2, T=2048, C=768, H=12,
inner=3072, fp32 I/O, causal, post-norm residual).

Sharding: 8 cores, token-interleaved. Core c handles batch c//4, tokens
p::4 (p = c%4) of that batch — every core runs the IDENTICAL program
(SPMD); causality is data-driven via per-core mask tensors.

v3 (vs 292us baseline): fused A' pipeline — scores+exp+ctx interleaved
INTO the K/V token-block loop so the ACT engine's ~60us of exp work (the
old phase-B bottleneck) hides under QKV matmuls:
- per tb: k, v computed; scores for the PREVIOUS tb's chunks run
  interleaved per head (score tiles -> exp -> boundary masks on the Pool
  engine -> ctx partial matmuls -> DVE accumulate into per-head f32
  SBUF accumulators ctxa [65,TQ]). PSUM: kv ring2 (2 banks) + v-ps2 (1)
  + score [128,1024] ring2 (4) + ctx [65,512] ring1 (1) = 8 exactly.
- q is kc-outer (6 psum banks, before the tb loop) with per-chunk
  wq/xtqh DMAs so the first matmul fires after ~330KB lands (was ~2MB).
- bv folded host-side into bo' = bo + bv @ Wo (ctx_norm = ctx*rcp
  absorbs +bv exactly via the denominator row): v eviction is a pure
  copy, no bias tile.
- softmax reciprocal = exp(-ln(dn)) on ACT (idle in B'), per head; the
  old 14.6us DVE reciprocal is gone. K=65 selector matmul broadcasts
  rcp; the psum->sbuf ctx eviction fuses into the normalize multiply.
- wk packed per-mc-slab so k(mc) streams as slabs land; wv split
  512/256; w1/w2 + xtq stream during B' when qz/kT/v free up.
"""

import sys

if "/opt/trn_rl_repo" not in sys.path:
    sys.path.insert(0, "/opt/trn_rl_repo")

import numpy as np
import ml_dtypes

import concourse.bacc as bacc
import concourse.mybir as mybir
import concourse.tile as tile
from concourse.bass_utils import run_bass_kernel_spmd

F32 = mybir.dt.float32
F32R = mybir.dt.float32r
BF16 = mybir.dt.bfloat16
ACTF = mybir.ActivationFunctionType

B, T, C = 2, 2048, 768
H, DH = 12, 64
IN = 3072
CC = C // 128          # 6 channel chunks
TBN = T // 512         # 4 token blocks of full seq
TQ = 512               # tokens per core
KCN = T // 128         # 16 k-chunks
ICN = IN // 128        # 24 inner chunks
EPS = 1e-4
SCALE = 1.0 / np.sqrt(DH)

# param pack order in "prk" [128, CC, 8]
P_BQ, P_BK, P_BO, P_B2, P_L1S, P_L1B, P_L2S, P_L2B = range(8)

# score-tile packing per tb: tiles of (global kc chunk, col offset).
# Chunk kc covers queries 32*kc..TQ (width TQ-32*kc). Offsets chosen so
# NO matmul output crosses a 2KB psum bank boundary (512 f32 cols) —
# a crossing write half-overwrites / half-accumulates-onto-stale.
# Gaps between chunks get exp'd (garbage, never read) — harmless.
SPACK = [
    [[(0, 0), (1, 512)], [(2, 0), (3, 512)]],
    [[(4, 0), (5, 512)], [(6, 0), (7, 512)]],
    [[(8, 0), (9, 256), (10, 512), (11, 704)]],
    [[(12, 0), (13, 128), (14, 224), (15, 288)]],
]


def _build_nc():
    nc = bacc.Bacc("TRN2", target_bir_lowering=False, debug=False,
                   enable_asserts=False, num_devices=8)
    d = {}
    d["xt"] = nc.dram_tensor("xt", [128, TBN, CC, 512], BF16,
                             kind="ExternalInput").ap()
    d["xtqh"] = nc.dram_tensor("xtqh", [128, CC, TQ], BF16,
                               kind="ExternalInput").ap()
    d["xtq"] = nc.dram_tensor("xtq", [128, CC, TQ], F32R,
                              kind="ExternalInput").ap()
    d["wq"] = nc.dram_tensor("wq", [128, CC, C], BF16,
                             kind="ExternalInput").ap()
    d["wk"] = nc.dram_tensor("wk", [128, CC, CC, 128], BF16,
                             kind="ExternalInput").ap()  # [p, mc, kc, j]
    d["wva"] = nc.dram_tensor("wva", [128, CC, 512], BF16,
                              kind="ExternalInput").ap()
    d["wvb"] = nc.dram_tensor("wvb", [128, CC, 256], BF16,
                              kind="ExternalInput").ap()
    d["wo"] = nc.dram_tensor("wo", [128, CC, C], BF16,
                             kind="ExternalInput").ap()
    d["w1"] = nc.dram_tensor("w1", [128, 4, CC, C], BF16,
                             kind="ExternalInput").ap()
    d["w2"] = nc.dram_tensor("w2", [128, 4, CC, C], BF16,
                             kind="ExternalInput").ap()
    d["prk"] = nc.dram_tensor("prk", [128, CC, 8], F32, kind="ExternalInput").ap()
    d["b1p"] = nc.dram_tensor("b1p", [128, ICN], F32, kind="ExternalInput").ap()
    d["msk"] = nc.dram_tensor("msk", [128, 32], BF16, kind="ExternalInput").ap()
    d["ones"] = nc.dram_tensor("ones", [128, H], F32R, kind="ExternalInput").ap()
    d["lnz"] = nc.dram_tensor("lnz", [2, 65, TQ], F32R, kind="ExternalInput").ap()
    d["selm"] = nc.dram_tensor("selm", [65, 3, DH], F32R, kind="ExternalInput").ap()
    d["lnt"] = nc.dram_tensor("lnt", [2, 65, C], F32R, kind="ExternalInput").ap()
    d["outT"] = nc.dram_tensor("outT", [C, TQ], F32, kind="ExternalOutput").ap()

    with tile.TileContext(nc) as tc:
        _emit(nc, tc, d)
    nc.finalize()
    return nc


def _ln_bcast(nc, pool, eps_sb, lnr1, lnr2, mean, ex2, tagp):
    """Fill lnr1 (row0 = istd) and lnr2 (row0 = mean*istd, row32 = ones)
    so the LN broadcasts run as K=65 selector matmuls at full PE rate.
    istd = exp(-0.5*ln(var+eps)) on ACT."""
    n = float(C)
    m2 = pool.tile([1, TQ], F32, name="ln_m2", tag=tagp + "m2")
    nc.vector.tensor_mul(m2[:], mean[:], mean[:])
    dv = pool.tile([1, TQ], F32, name="ln_d", tag=tagp + "d")
    nc.vector.tensor_sub(dv[:], ex2[:], m2[:])
    lnv = pool.tile([1, TQ], F32, name="ln_lnv", tag=tagp + "lnv")
    nc.scalar.activation(lnv[:], dv[:], ACTF.Ln,
                         scale=n / (n - 1.0), bias=eps_sb[:])
    with nc.allow_low_precision(reason="f32r matmul operand"):
        nc.scalar.activation(lnr1[0:1, :], lnv[:], ACTF.Exp, scale=-0.5)
        nc.vector.tensor_mul(lnr2[0:1, :], mean[:], lnr1[0:1, :])


def _emit(nc, tc, d):
    # ---- persistent constants ------------------------------------------
    const = tc.alloc_tile_pool(name="const", bufs=1, side="left")
    onesh_sb = const.tile([128, H], F32R, name="onesh_sb")
    selm_sb = const.tile([65, 3, DH], F32R, name="selm_sb")
    lnt1_sb = const.tile([65, C], F32R, name="lnt1_sb")
    lnt2_sb = const.tile([65, C], F32R, name="lnt2_sb")
    eps_sb = const.tile([1, 1], F32, name="eps_sb")
    prk_sb = const.tile([128, CC, 8], F32, name="prk_sb")
    b1p_sb = const.tile([128, ICN], F32, name="b1p_sb")
    mskb_sb = const.tile([128, 32], BF16, name="mskb_sb")

    def prm(cc, pi):
        return prk_sb[:, cc, pi].unsqueeze(-1)  # [128,1]

    nc.vector.memset(eps_sb[:], float(EPS))
    nc.gpsimd.dma_start(out=onesh_sb[:], in_=d["ones"][:])
    ones1_sb = onesh_sb[:, 0:1]
    nc.gpsimd.dma_start(out=prk_sb[:], in_=d["prk"][:])
    nc.gpsimd.dma_start(out=mskb_sb[:], in_=d["msk"][:])
    nc.gpsimd.dma_start(out=selm_sb[:], in_=d["selm"][:])
    nc.gpsimd.dma_start(out=lnt1_sb[:], in_=d["lnt"][0, :, :])
    nc.gpsimd.dma_start(out=lnt2_sb[:], in_=d["lnt"][1, :, :])
    nc.gpsimd.dma_start(out=b1p_sb[:], in_=d["b1p"][:])

    # ---- persistent activation tensors ---------------------------------
    # left-side stack order = reverse release order: ctxT/ctxa released
    # late, qT/kv right after A'
    ctxT_pool = tc.alloc_tile_pool(name="ctxTp", bufs=1, side="left")
    ctxT_sb = [ctxT_pool.tile([128, TQ], BF16, name=f"ctxT{cc}")
               for cc in range(CC)]
    ctxa_pool = tc.alloc_tile_pool(name="ctxap", bufs=1, side="left")
    ctxa_sb = [ctxa_pool.tile([DH + 1, TQ], F32, name=f"ctxa{h}")
               for h in range(H)]
    qT_pool = tc.alloc_tile_pool(name="qTp", bufs=1, side="left")
    qz_sb = [qT_pool.tile([128, TQ], BF16, name=f"qz{h}") for h in range(H)]
    for h in range(H):
        ro = (h % 2) * DH
        with nc.allow_low_precision(reason="zero fill"):
            nc.vector.memset(qz_sb[h][(DH - ro):(128 - ro), :], 0.0)

    kv_pool = tc.alloc_tile_pool(name="kvp", bufs=1, side="left")
    kT_sb = [kv_pool.tile([128, T], BF16, name=f"kT{cc}") for cc in range(CC)]
    v_sb = [kv_pool.tile([128, H, DH + 1], BF16, name=f"v{tch}")
            for tch in range(KCN)]
    for tch in range(KCN):
        with nc.allow_low_precision(reason="ones fill"):
            nc.vector.tensor_copy(v_sb[tch][:, :, DH], onesh_sb[:])

    # ==================== phase A': QKV + scores + exp + ctx ============
    wo_pool = tc.alloc_tile_pool(name="wop", bufs=1, side="right")
    with tc.tile_pool(name="xqs", bufs=1, side="right") as xq_pool, \
         tc.tile_pool(name="wqs", bufs=1, side="right") as wq_pool, \
         tc.tile_pool(name="wkvs", bufs=1, side="right") as wkv_pool, \
         tc.tile_pool(name="xts", bufs=2, side="right") as xt_pool, \
         tc.tile_pool(name="etp", bufs=1, side="right") as et_pool:

        # -- DMA schedule ------------------------------------------------
        # sync q: xtqh chunks, xt blocks, wva/wvb
        # scalar q: wq chunks, wk mc-slabs
        # gpsimd q: small consts (above), wo
        xtqh_pk = xq_pool.tile([128, CC, TQ], BF16, name="xtqh_pk")
        wq_pk = wq_pool.tile([128, CC, C], BF16, name="wq_pk")
        for kc in range(CC):
            nc.sync.dma_start(out=xtqh_pk[:, kc, :], in_=d["xtqh"][:, kc, :])
            nc.scalar.dma_start(out=wq_pk[:, kc, :], in_=d["wq"][:, kc, :])
        wk_pk = wkv_pool.tile([128, CC, CC, 128], BF16, name="wk_pk")
        for mc in range(CC):
            nc.scalar.dma_start(out=wk_pk[:, mc, :, :],
                                in_=d["wk"][:, mc, :, :])
        xt_tiles = {}
        for tb in range(2):
            t = xt_pool.tile([128, CC, 512], BF16, name="xt_t", tag="xt")
            nc.sync.dma_start(out=t[:], in_=d["xt"][:, tb, :, :])
            xt_tiles[tb] = t
        wva_pk = wkv_pool.tile([128, CC, 512], BF16, name="wva_pk")
        nc.gpsimd.dma_start(out=wva_pk[:], in_=d["wva"][:])
        wvb_pk = wkv_pool.tile([128, CC, 256], BF16, name="wvb_pk")
        nc.gpsimd.dma_start(out=wvb_pk[:], in_=d["wvb"][:])
        wo_pk = wo_pool.tile([128, CC, C], BF16, name="wo_pk")
        nc.gpsimd.dma_start(out=wo_pk[:], in_=d["wo"][:])
        wo_sb = [wo_pk[:, cc, :] for cc in range(CC)]

        # -- q: kc-outer so first matmul needs only chunk 0 of wq/xtqh --
        with tc.tile_pool(name="pq", bufs=1, space="PSUM") as pq:
            ps_q = [pq.tile([128, TQ], F32, name=f"psq{mc}", tag=f"q{mc}")
                    for mc in range(CC)]
            for kc in range(CC):
                for mc in range(CC):
                    nc.tensor.matmul(ps_q[mc][:],
                                     wq_pk[:, kc, mc * 128:(mc + 1) * 128],
                                     xtqh_pk[:, kc, :],
                                     start=(kc == 0), stop=(kc == CC - 1))
            for mc in range(CC):
                with nc.allow_low_precision(reason="bf16 activations"):
                    nc.vector.tensor_scalar_add(
                        qz_sb[2 * mc][0:DH, :], ps_q[mc][0:DH, :],
                        prm(mc, P_BQ)[0:DH, :])
                    nc.vector.tensor_scalar_add(
                        qz_sb[2 * mc + 1][DH:128, :], ps_q[mc][DH:128, :],
                        prm(mc, P_BQ)[DH:128, :])

        with tc.tile_pool(name="psc", bufs=1, space="PSUM") as psc, \
             tc.tile_pool(name="pctx", bufs=1, space="PSUM") as pctx:

            cur_et = {}   # h -> list of (et_tile, off, w, kc) for this tb

            def emit_scores(tb, h):
                cc = h // 2
                kTh = kT_sb[cc]
                qzh = qz_sb[h]
                lst = []
                for kcs in SPACK[tb]:
                    tw = kcs[-1][1] + TQ - 32 * kcs[-1][0]
                    ps = psc.tile([128, 1024], F32, name="ps_s",
                                  tag="sA", bufs=2)
                    for kc, off in kcs:
                        nc.tensor.matmul(
                            ps[:, off:off + TQ - 32 * kc],
                            kTh[:, kc * 128:(kc + 1) * 128],
                            qzh[:, 32 * kc:TQ],
                            start=True, stop=True)
                    et = et_pool.tile([128, 1024], BF16, name="et",
                                      tag="et", bufs=6)
                    nc.scalar.activation(et[:, 0:tw], ps[:, 0:tw], ACTF.Exp,
                                         scale=float(SCALE))
                    for kc, o in kcs:
                        with nc.allow_low_precision(reason="bf16 mask"):
                            nc.vector.tensor_mul(
                                et[:, o:o + 32], et[:, o:o + 32],
                                mskb_sb[:])
                        lst.append((et, o, TQ - 32 * kc, kc))
                cur_et[h] = lst

            def emit_ctx(tb, h):
                ps = pctx.tile([DH + 1, TQ], F32, name="ctx_ps", tag="ctx",
                               bufs=1)
                lst = cur_et.pop(h)
                w0 = lst[0][2]
                for i, (et, o, w, kc) in enumerate(lst):
                    nc.tensor.matmul(ps[:, TQ - w:], v_sb[kc][:, h, :],
                                     et[:, o:o + w],
                                     start=(i == 0), stop=(i == len(lst) - 1))
                if tb == 0:
                    nc.vector.tensor_copy(ctxa_sb[h][:], ps[:])
                else:
                    nc.vector.tensor_add(ctxa_sb[h][:, TQ - w0:],
                                         ctxa_sb[h][:, TQ - w0:],
                                         ps[:, TQ - w0:])

            # windows 0-2: scores/exp/ctx of tb interleaved with k/v(tb+1)
            with tc.tile_pool(name="pkv", bufs=1, space="PSUM") as pkv:

                def emit_k(tb, mc):
                    xt_blk = xt_tiles[tb]
                    ps = pkv.tile([128, 512], F32, name="ps_k", tag="kv1",
                                  bufs=2)
                    for kc in range(CC):
                        nc.tensor.matmul(
                            ps[:], wk_pk[:, mc, kc, :], xt_blk[:, kc, :],
                            start=(kc == 0), stop=(kc == CC - 1))
                    with nc.allow_low_precision(reason="bf16 activations"):
                        nc.vector.tensor_scalar_add(
                            kT_sb[mc][:, tb * 512:(tb + 1) * 512], ps[:],
                            prm(mc, P_BK))

                def emit_v(tb, tci):
                    xt_blk = xt_tiles[tb]
                    tch = tb * 4 + tci
                    ps1 = pkv.tile([128, 512], F32, name="ps_v1", tag="kv1",
                                   bufs=2)
                    ps2 = pkv.tile([128, 256], F32, name="ps_v2", tag="v2",
                                   bufs=1)
                    for kc in range(CC):
                        xsl = xt_blk[:, kc, tci * 128:(tci + 1) * 128]
                        nc.tensor.matmul(ps1[:], xsl, wva_pk[:, kc, :],
                                         start=(kc == 0), stop=(kc == CC - 1))
                        nc.tensor.matmul(ps2[:], xsl, wvb_pk[:, kc, :],
                                         start=(kc == 0), stop=(kc == CC - 1))
                    vt = v_sb[tch]
                    with nc.allow_low_precision(reason="bf16 activations"):
                        nc.vector.tensor_copy(
                            vt[:, 0:8, 0:DH],
                            ps1[:].rearrange("p (h d) -> p h d", d=DH))
                        nc.vector.tensor_copy(
                            vt[:, 8:H, 0:DH],
                            ps2[:].rearrange("p (h d) -> p h d", d=DH))

                # tb0 k/v (nothing to interleave yet)
                for mc in range(CC):
                    emit_k(0, mc)
                for tci in range(4):
                    emit_v(0, tci)

                for stb in range(TBN - 1):
                    ftb = stb + 1
                    if ftb + 1 < TBN:
                        t2 = xt_pool.tile([128, CC, 512], BF16,
                                          name="xt_t", tag="xt")
                        nc.sync.dma_start(out=t2[:],
                                          in_=d["xt"][:, ftb + 1, :, :])
                        xt_tiles[ftb + 1] = t2
                        del t2
                    fills = [(emit_k, (ftb, mc)) for mc in range(CC)]
                    fills += [(emit_v, (ftb, tci)) for tci in range(4)]
                    for h in range(H):
                        emit_scores(stb, h)
                        if h > 0:
                            emit_ctx(stb, h - 1)
                        if h < len(fills):
                            fn, args = fills[h]
                            fn(*args)
                    emit_ctx(stb, H - 1)

            # window 3: tb3 scores/ctx with softmax normalization fused in
            # (kv banks freed above; pb broadcasts use them). rcp on DVE —
            # ln/exp tables are in different ACT sets, a per-head ln/exp
            # reciprocal costs 2 table loads (1.28us each).
            with tc.tile_pool(name="ppb", bufs=1, space="PSUM") as ppb:
                dnb = [et_pool.tile([65, TQ], F32, name=f"dnb{i}")
                       for i in range(2)]
                nc.vector.memset(dnb[0][:], 1.0)
                nc.vector.memset(dnb[1][:], 1.0)

                def emit_norm(bi):
                    dnt = dnb[bi % 2]
                    for j in range(3):
                        h = 3 * bi + j
                        nc.vector.tensor_copy(dnt[32 * j:32 * j + 1, :],
                                              ctxa_sb[h][DH:DH + 1, :])
                    rcpf = et_pool.tile([65, TQ], F32, name="rcpf",
                                        tag="rcpf", bufs=2)
                    nc.vector.reciprocal_approx_fast(rcpf[:], dnt[:])
                    rcp = et_pool.tile([65, TQ], F32R, name="rcp",
                                       tag="rcp", bufs=2)
                    with nc.allow_low_precision(reason="f32r operand"):
                        nc.vector.tensor_copy(rcp[:], rcpf[:])
                    for j in range(3):
                        h = 3 * bi + j
                        cc, ro = h // 2, (h % 2) * DH
                        pb = ppb.tile([DH, TQ], F32, name="pb", tag="pb",
                                      bufs=2)
                        nc.tensor.matmul(pb[:], selm_sb[:, j, :], rcp[:],
                                         start=True, stop=True)
                        with nc.allow_low_precision(reason="bf16 out"):
                            nc.vector.tensor_mul(
                                ctxT_sb[cc][ro:ro + DH, :],
                                ctxa_sb[h][0:DH, :], pb[:])

                for h in range(H):
                    emit_scores(3, h)
                    if h > 0:
                        emit_ctx(3, h - 1)
                    if h in (4, 7, 10):
                        emit_norm((h - 4) // 3)
                emit_ctx(3, H - 1)
                emit_norm(3)

    kv_pool.release()
    qT_pool.release()

    # ==================== phase B': norm + Wo + LN1 =====================
    w1pool = tc.alloc_tile_pool(name="w1pool", bufs=2, side="right")
    w2pool = tc.alloc_tile_pool(name="w2pool", bufs=2, side="right")
    xtq_pool = tc.alloc_tile_pool(name="xtqp", bufs=1, side="right")

    hT_holder = {}
    with tc.tile_pool(name="bpool", bufs=2, side="right") as bpool, \
         tc.tile_pool(name="r1pool", bufs=1, side="right") as r1pool:

        # streams that waited on qz/kT/v SBUF space
        xtq_pk = xtq_pool.tile([128, CC, TQ], F32R, name="xtq_pk")
        nc.sync.dma_start(out=xtq_pk[:], in_=d["xtq"][:])
        xtq_sb = [xtq_pk[:, cc, :] for cc in range(CC)]
        w1blk = {}
        w2g = {}
        t = w2pool.tile([128, CC, C], BF16, name="w2_t", tag="w2")
        nc.sync.dma_start(out=t[:], in_=d["w2"][:, 0, :, :])
        w2g[0] = t
        for jb in range(2):
            t = w1pool.tile([128, CC, C], BF16, name="w1_t", tag="w1")
            nc.sync.dma_start(out=t[:], in_=d["w1"][:, jb, :, :])
            w1blk[jb] = [t[:, kc, :] for kc in range(CC)]
        t = w2pool.tile([128, CC, C], BF16, name="w2_t", tag="w2")
        nc.sync.dma_start(out=t[:], in_=d["w2"][:, 1, :, :])
        w2g[1] = t

        # preload the LN act table off the critical LN1 chain
        dmy = bpool.tile([1, 1], F32, name="dmy", tag="dmy")
        nc.scalar.activation(dmy[:], eps_sb[:], ACTF.Ln)

        # Wo + residual + LN1 stats
        r1_sb = [r1pool.tile([128, TQ], F32R, name=f"r1{cc}")
                 for cc in range(CC)]
        lnr1 = bpool.tile([65, TQ], F32R, name="lnr1", tag="lnr1")
        lnr2 = bpool.tile([65, TQ], F32R, name="lnr2", tag="lnr2")
        nc.sync.dma_start(out=lnr1[:], in_=d["lnz"][0, :, :])
        nc.sync.dma_start(out=lnr2[:], in_=d["lnz"][1, :, :])
        with tc.tile_pool(name="pao", bufs=2, space="PSUM") as pao, \
             tc.tile_pool(name="pst", bufs=2, space="PSUM") as pst:
            ps_sum = pst.tile([1, TQ], F32, name="ps_sum", tag="st")
            ps_sq = pst.tile([1, TQ], F32, name="ps_sq", tag="st")
            for mc in range(CC):
                ps = pao.tile([128, TQ], F32, name="ps_ao", tag="ao")
                for kc in range(CC):
                    nc.tensor.matmul(ps[:],
                                     wo_sb[kc][:, mc * 128:(mc + 1) * 128],
                                     ctxT_sb[kc][:],
                                     start=(kc == 0), stop=(kc == CC - 1))
                nc.vector.scalar_tensor_tensor(
                    r1_sb[mc][:], ps[:], prm(mc, P_BO), xtq_sb[mc][:],
                    mybir.AluOpType.add, mybir.AluOpType.add)
                nc.tensor.matmul(ps_sum[:], ones1_sb[:], r1_sb[mc][:],
                                 start=(mc == 0), stop=(mc == CC - 1))
                sq = bpool.tile([128, TQ], F32R, name="sq", tag="sq")
                nc.scalar.activation(sq[:], r1_sb[mc][:], ACTF.Square)
                nc.tensor.matmul(ps_sq[:], ones1_sb[:], sq[:],
                                 start=(mc == 0), stop=(mc == CC - 1))
            n = float(C)
            mean1 = bpool.tile([1, TQ], F32R, name="l1mean", tag="l1mean")
            nc.scalar.activation(mean1[:], ps_sum[:], ACTF.Copy, scale=1.0 / n)
            ex21 = bpool.tile([1, TQ], F32, name="l1ex2", tag="l1ex2")
            nc.scalar.activation(ex21[:], ps_sq[:], ACTF.Copy, scale=1.0 / n)
        hT_pool = tc.alloc_tile_pool(name="hTp", bufs=1, side="left")
        hT_sb = [hT_pool.tile([128, TQ], BF16, name=f"hT{cc}")
                 for cc in range(CC)]
        hT_holder["pool"] = hT_pool
        _ln_bcast(nc, bpool, eps_sb, lnr1, lnr2, mean1, ex21, "l1")
        nc.scalar.activation(dmy[:], eps_sb[:], ACTF.Gelu_apprx_tanh)
        with tc.tile_pool(name="pbc2", bufs=2, space="PSUM") as pbc2:
            for cc in range(CC):
                csl = slice(cc * 128, (cc + 1) * 128)
                pb2 = pbc2.tile([128, 2 * TQ], F32, name="lnpb", tag="bc")
                nc.tensor.matmul(pb2[:, 0:TQ], lnt1_sb[:, csl],
                                 lnr1[:], start=True, stop=True)
                nc.tensor.matmul(pb2[:, TQ:], lnt1_sb[:, csl],
                                 lnr2[:], start=True, stop=True)
                t1 = bpool.tile([128, TQ], F32, name="ln_t1", tag="lnt1")
                nc.vector.tensor_mul(t1[:], r1_sb[cc][:], pb2[:, 0:TQ])
                with nc.allow_low_precision(reason="bf16 activations"):
                    nc.vector.tensor_sub(hT_sb[cc][:], t1[:], pb2[:, TQ:])

    xtq_pool.release()

    # ==================== phase D: MLP + residual + LN2 =================
    with tc.tile_pool(name="dpool", bufs=2, side="right") as dpool, \
         tc.tile_pool(name="r2pool", bufs=1, side="right") as r2pool:

        r2_sb = [r2pool.tile([128, TQ], F32R, name=f"r2{cc}")
                 for cc in range(CC)]
        lnr1b = dpool.tile([65, TQ], F32R, name="lnr1b", tag="lnr1b")
        lnr2b = dpool.tile([65, TQ], F32R, name="lnr2b", tag="lnr2b")
        nc.sync.dma_start(out=lnr1b[:], in_=d["lnz"][0, :, :])
        nc.sync.dma_start(out=lnr2b[:], in_=d["lnz"][1, :, :])
        with tc.tile_pool(name="pfc2", bufs=1, space="PSUM") as pfc2:
            ps_m = [pfc2.tile([128, TQ], F32, name=f"ps_m{mc}", tag=f"m{mc}")
                    for mc in range(CC)]
            with tc.tile_pool(name="pfc1", bufs=2, space="PSUM") as pfc1:
                for kc2 in range(ICN):
                    jb = kc2 // CC
                    ps1 = pfc1.tile([128, TQ], F32, name="ps1", tag="f1")
                    co = (kc2 % CC) * 128
                    for kc in range(CC):
                        nc.tensor.matmul(
                            ps1[:], w1blk[jb][kc][:, co:co + 128],
                            hT_sb[kc][:],
                            start=(kc == 0), stop=(kc == CC - 1))
                    g = dpool.tile([128, TQ], BF16, name="g", tag="g")
                    with nc.allow_low_precision(reason="bf16 activations"):
                        nc.scalar.activation(g[:], ps1[:],
                                             ACTF.Gelu_apprx_tanh,
                                             bias=b1p_sb[:, kc2].unsqueeze(-1))
                    w2t = w2g[kc2 // CC][:, kc2 % CC, :]
                    for mc in range(CC):
                        nc.tensor.matmul(ps_m[mc][:],
                                         w2t[:, mc * 128:(mc + 1) * 128],
                                         g[:], start=(kc2 == 0),
                                         stop=(kc2 == ICN - 1))
                    # ring prefetches: issued after this iteration's readers
                    if kc2 % CC == CC - 1 and jb + 2 <= 3:
                        t = w1pool.tile([128, CC, C], BF16, name="w1_t",
                                        tag="w1")
                        nc.sync.dma_start(out=t[:],
                                          in_=d["w1"][:, jb + 2, :, :])
                        w1blk[jb + 2] = [t[:, kc, :] for kc in range(CC)]
                        t2 = w2pool.tile([128, CC, C], BF16, name="w2_t",
                                         tag="w2")
                        nc.sync.dma_start(out=t2[:],
                                          in_=d["w2"][:, jb + 2, :, :])
                        w2g[jb + 2] = t2
            dmy2 = dpool.tile([1, 1], F32, name="dmy2", tag="dmy2")
            nc.scalar.activation(dmy2[:], eps_sb[:], ACTF.Ln)
            with tc.tile_pool(name="pst2", bufs=2, space="PSUM") as pst2:
                ps_sum2 = pst2.tile([1, TQ], F32, name="ps_sum2", tag="st")
                ps_sq2 = pst2.tile([1, TQ], F32, name="ps_sq2", tag="st")
                for mc in range(CC):
                    nc.vector.scalar_tensor_tensor(
                        r2_sb[mc][:], ps_m[mc][:], prm(mc, P_B2),
                        hT_sb[mc][:], mybir.AluOpType.add,
                        mybir.AluOpType.add)
                    nc.tensor.matmul(ps_sum2[:], ones1_sb[:], r2_sb[mc][:],
                                     start=(mc == 0), stop=(mc == CC - 1))
                    sq = dpool.tile([128, TQ], F32R, name="sq2", tag="sq")
                    nc.scalar.activation(sq[:], r2_sb[mc][:], ACTF.Square)
                    nc.tensor.matmul(ps_sq2[:], ones1_sb[:], sq[:],
                                     start=(mc == 0), stop=(mc == CC - 1))
                n = float(C)
                mean2 = dpool.tile([1, TQ], F32R, name="l2mean", tag="l2mean")
                nc.scalar.activation(mean2[:], ps_sum2[:], ACTF.Copy,
                                     scale=1.0 / n)
                ex22 = dpool.tile([1, TQ], F32, name="l2ex2", tag="l2ex2")
                nc.scalar.activation(ex22[:], ps_sq2[:], ACTF.Copy,
                                     scale=1.0 / n)
        hT_holder["pool"].release()
        ctxa_pool.release()
        _ln_bcast(nc, dpool, eps_sb, lnr1b, lnr2b, mean2, ex22, "l2")
        with tc.tile_pool(name="pbc3", bufs=2, space="PSUM") as pbc3:
            for cc in range(CC):
                csl = slice(cc * 128, (cc + 1) * 128)
                pb3 = pbc3.tile([128, 2 * TQ], F32, name="lnpb3", tag="bc")
                nc.tensor.matmul(pb3[:, 0:TQ], lnt2_sb[:, csl],
                                 lnr1b[:], start=True, stop=True)
                nc.tensor.matmul(pb3[:, TQ:], lnt2_sb[:, csl],
                                 lnr2b[:], start=True, stop=True)
                t1 = dpool.tile([128, TQ], F32, name="ln_t13", tag="lnt13")
                nc.vector.tensor_mul(t1[:], r2_sb[cc][:], pb3[:, 0:TQ])
                ot = dpool.tile([128, TQ], F32, name=f"o{cc}", tag=f"o{cc}",
                                bufs=1)
                nc.vector.tensor_sub(ot[:], t1[:], pb3[:, TQ:])
                nc.sync.dma_start(out=d["outT"][cc * 128:(cc + 1) * 128, :],
                                  in_=ot[:])

    w2pool.release()
    w1pool.release()
    wo_pool.release()
    ctxT_pool.release()
    const.release()


_NC = None


def _get_nc():
    global _NC
    if _NC is None:
        _NC = _build_nc()
    return _NC


def _prep_inmaps(x, Wq, bq, Wk, bk, Wv, bv, Wo, bo, ln1_s, ln1_b,
                 W1, b1, W2, b2, ln2_s, ln2_b):
    f32 = np.float32
    bf16 = ml_dtypes.bfloat16

    def pk(a):
        # [A*128, c...] -> [128, A, c...] contiguous (partition-major)
        a = np.asarray(a)
        return np.ascontiguousarray(
            a.reshape(-1, 128, *a.shape[1:]).swapaxes(0, 1))

    xT = [np.ascontiguousarray(np.asarray(x)[b].T, dtype=f32)
          for b in range(B)]
    xTh = [xb.astype(bf16) for xb in xT]
    wq = pk(np.asarray(Wq, f32).astype(bf16))
    # wk packed per-mc-slab: [128, mc, kc, 128]
    wk_p = pk(np.asarray(Wk, f32).astype(bf16))        # [128, kc, 768]
    wk = np.ascontiguousarray(
        wk_p.reshape(128, CC, CC, 128).swapaxes(1, 2))
    wv_p = pk(np.asarray(Wv, f32).astype(bf16))
    wva = np.ascontiguousarray(wv_p[:, :, 0:512])
    wvb = np.ascontiguousarray(wv_p[:, :, 512:768])
    wo = pk(np.asarray(Wo, f32).astype(bf16))
    w1f = np.asarray(W1, f32).astype(bf16)
    w1 = np.stack([pk(w1f[:, jb * C:(jb + 1) * C]) for jb in range(4)],
                  axis=1)
    w2f = np.asarray(W2, f32).astype(bf16)
    w2 = np.stack([pk(w2f[gg * C:(gg + 1) * C, :]) for gg in range(4)],
                  axis=1)
    # fold bv into bo: ctx_norm = ctx*rcp absorbs +bv exactly
    bo_f = np.asarray(bo, f32) + np.asarray(bv, f32) @ np.asarray(Wo, f32)
    prk = np.zeros((128, CC, 8), f32)
    for pi, arr in ((P_BQ, bq), (P_BK, bk), (P_BO, bo_f), (P_B2, b2),
                    (P_L1S, ln1_s), (P_L1B, ln1_b), (P_L2S, ln2_s),
                    (P_L2B, ln2_b)):
        prk[:, :, pi] = np.asarray(arr, f32).reshape(CC, 128).T
    b1p = np.ascontiguousarray(np.asarray(b1, f32).reshape(ICN, 128).T)
    lnt = np.zeros((2, 65, C), f32)
    lnt[0, 0], lnt[0, 32] = np.asarray(ln1_s, f32), -np.asarray(ln1_b, f32)
    lnt[1, 0], lnt[1, 32] = np.asarray(ln2_s, f32), -np.asarray(ln2_b, f32)
    selm = np.zeros((65, 3, DH), f32)
    for j in range(3):
        selm[32 * j, j, :] = 1.0
    ones = np.ones((128, H), f32)
    lnz = np.zeros((2, 65, TQ), f32)
    lnz[1, 32, :] = 1.0

    kk = np.arange(128)[:, None]
    in_maps = []
    for c in range(8):
        b, p = c // 4, c % 4
        qq = np.arange(32)[None, :]
        msk = np.ascontiguousarray(
            (kk <= 4 * qq + p).astype(bf16))  # k <= 4j+p, all chunks

        xtp = pk(xTh[b])  # [128, CC, T]
        xtp = np.ascontiguousarray(
            xtp.reshape(128, CC, TBN, 512).swapaxes(1, 2))
        in_maps.append({
            "xt": xtp,
            "xtqh": pk(np.ascontiguousarray(xTh[b][:, p::4])),
            "xtq": pk(np.ascontiguousarray(xT[b][:, p::4])),
            "wq": wq, "wk": wk, "wva": wva, "wvb": wvb, "wo": wo,
            "w1": w1, "w2": w2,
            "prk": prk, "b1p": b1p, "msk": msk,
            "lnt": lnt, "selm": selm, "ones": ones, "lnz": lnz,
        })
    return in_maps


def _run(in_maps, trace=False, **kw):
    nc = _get_nc()
    return run_bass_kernel_spmd(nc, in_maps, list(range(8)), trace=trace, **kw)


def kernel(**inputs):
    in_maps = _prep_inmaps(**inputs)
    res = _run(in_maps)
    out = np.empty((B, T, C), np.float32)
    for c in range(8):
        b, p = c // 4, c % 4
        out[b, p::4, :] = res.results[c]["outT"].T
    return out
